# revision 1
# baseline (speedup 1.0000x reference)
"""Trainium2 Bass kernel for nn_BlockModel_82678120448388.

Model: per (batch, head): 8x8 transition matrices from an MLP (normalized),
values from a second MLP, then a linear recurrence s_t = A_t s_{t-1} + v_t
over seq=2048.

Sharding: 8 cores = 4 batches x 2 head-halves (32 heads each). Weights
replicated / row-sliced on host; full inputs in, full output out.
"""

import numpy as np
import ml_dtypes
from contextlib import ExitStack

import concourse.bass as bass
import concourse.bacc as bacc
import concourse.tile as tile
from concourse import mybir

F32 = mybir.dt.float32
BF16 = mybir.dt.bfloat16
AF = mybir.ActivationFunctionType
ALU = mybir.AluOpType

BS, SEQ, EMB, BD = 4, 2048, 512, 8
H = EMB // BD      # 64 global heads
HL = 32            # heads per core
NF = HL * BD * BD  # 2048 blk feats per core
VF = HL * BD       # 256 v feats per core
HID = EMB * BD     # 4096
P = 128
JW = BD + 1        # augmented [T|u] column count

N_CORES = 8


def build_nc(TOK=SEQ, K=16, p1_steps=None, pc_steps=None, nq_steps=None):
    """Per-core Bass module. TOK tokens, K chunks (chunk len C=TOK//K)."""
    C = TOK // K
    QT = min(512, TOK)     # L1 token-chunk
    NQ = TOK // QT
    TPQ = QT // P          # tok-tiles per q
    NHO = P // K           # head-groups per chunk on partitions (8 for K=16)
    NHR = HL // NHO        # heads per group in free dim (4)
    HRI = NHR * BD         # 32

    assert TOK % QT == 0 and QT % P == 0 and P % K == 0

    nc = bacc.Bacc("TRN2", target_bir_lowering=False, debug=False)

    xT = nc.dram_tensor("xT", [EMB, TOK], BF16, kind="ExternalInput")
    w1 = nc.dram_tensor("w1", [EMB, HID], BF16, kind="ExternalInput")
    b1 = nc.dram_tensor("b1", [HID, 1], F32, kind="ExternalInput")
    w2 = nc.dram_tensor("w2", [HID, NF], BF16, kind="ExternalInput")
    b2 = nc.dram_tensor("b2", [1, NF], BF16, kind="ExternalInput")
    v1 = nc.dram_tensor("v1", [EMB, EMB], BF16, kind="ExternalInput")
    c1 = nc.dram_tensor("c1", [EMB, 1], F32, kind="ExternalInput")
    v2 = nc.dram_tensor("v2", [EMB, VF], BF16, kind="ExternalInput")
    c2 = nc.dram_tensor("c2", [1, VF], BF16, kind="ExternalInput")
    a0 = nc.dram_tensor("a0", [NHO, HRI], F32, kind="ExternalInput")
    smat = nc.dram_tensor("smat", [P, P], F32, kind="ExternalInput")
    tinit = nc.dram_tensor("tinit", [P, K * JW], F32, kind="ExternalInput")
    out = nc.dram_tensor("out", [TOK, VF], F32, kind="ExternalOutput")

    a_dram = nc.dram_tensor("a_scratch", [TOK, NF], F32)
    tst_dram = nc.dram_tensor("tst_scratch", [2 * P, K * JW], F32)
    v_dram = nc.dram_tensor("v_scratch", [TOK, VF], F32)

    with ExitStack() as ctx:
        tc = ctx.enter_context(tile.TileContext(nc))
        cpool = ctx.enter_context(tc.tile_pool(name="consts", bufs=1))
        wpool = ctx.enter_context(tc.tile_pool(name="weights", bufs=1))
        xpool = ctx.enter_context(tc.tile_pool(name="xstream", bufs=2))
        hpool = ctx.enter_context(tc.tile_pool(name="hidden", bufs=1))
        w2pool = ctx.enter_context(tc.tile_pool(name="w2stream", bufs=4))
        l1ps = ctx.enter_context(tc.tile_pool(name="l1ps", bufs=2, space="PSUM"))
        p1ps = ctx.enter_context(tc.tile_pool(name="p1ps", bufs=2, space="PSUM"))
        l2ps = ctx.enter_context(tc.tile_pool(name="l2ps", bufs=TPQ, space="PSUM"))
        vps = ctx.enter_context(tc.tile_pool(name="vps", bufs=1, space="PSUM"))
        blkpool = ctx.enter_context(tc.tile_pool(name="blk", bufs=TPQ + 1))
        pwpool = ctx.enter_context(tc.tile_pool(name="pw", bufs=2))
        smpool = ctx.enter_context(tc.tile_pool(name="small", bufs=3))
        vtpool = ctx.enter_context(tc.tile_pool(name="vtile", bufs=2))
        agpool = ctx.enter_context(tc.tile_pool(name="agather", bufs=3))
        vgpool = ctx.enter_context(tc.tile_pool(name="vgather", bufs=3))
        mopool = ctx.enter_context(tc.tile_pool(name="multout", bufs=3))
        tupool = ctx.enter_context(tc.tile_pool(name="tu", bufs=2))
        scpool = ctx.enter_context(tc.tile_pool(name="scan", bufs=1))

        # ---- constants / weights ----
        ones_s = cpool.tile([1, P], BF16, tag="ones")
        nc.vector.memset(ones_s[:], 1.0)
        b1_s = cpool.tile([P, HID // P], F32, tag="b1")
        nc.sync.dma_start(b1_s[:], b1[:].rearrange("(m p) one -> p (m one)", p=P))
        c1_s = cpool.tile([P, EMB // P], F32, tag="c1")
        nc.sync.dma_start(c1_s[:], c1[:].rearrange("(m p) one -> p (m one)", p=P))
        b2_s = cpool.tile([1, NF], BF16, tag="b2")
        nc.sync.dma_start(b2_s[:], b2[:])
        c2_s = cpool.tile([1, VF], BF16, tag="c2")
        nc.sync.dma_start(c2_s[:], c2[:])
        a0_s = cpool.tile([NHO, HRI], F32, tag="a0")
        nc.sync.dma_start(a0_s[:], a0[:])
        smat_s = cpool.tile([P, P], F32, tag="smat")
        nc.sync.dma_start(smat_s[:], smat[:])

        v1_s = wpool.tile([P, 4, EMB], BF16, tag="v1")
        nc.sync.dma_start(v1_s[:], v1[:].rearrange("(k p) m -> p k m", p=P))
        v2_s = wpool.tile([P, 4, VF], BF16, tag="v2")
        nc.sync.dma_start(v2_s[:], v2[:].rearrange("(k p) n -> p k n", p=P))

        # ================= scan helpers =================
        # a_dram row tau*128 + c*8 + j holds token c*C + 8*tau + j, feats in
        # (head, col, row) order. Phase 1 layout: partition = (hpack16, k8),
        # Tst[(h,k), (c, j9)] = [T|u][row k, col j] for chunk c; two packs.
        TUP = NHR * BD * JW  # 288 (old layout, used by phase B/C)
        TSP = K * JW         # 144 Tst row size

        def rowbase(r):
            tau, j = r // 8, r % 8
            return tau * P + j

        tu_box = {}

        def g_A(r):
            ag = agpool.tile([P, HL * BD], F32, tag="ag", name=f"ag{r}")
            nc.sync.dma_start(ag[:], bass.AP(
                a_dram, rowbase(r) * NF,
                [[8 * NF, K], [NHR * BD * BD, NHO], [1, NHR * BD * BD]]))
            return ag

        def g_v(r):
            vg = vgpool.tile([P, HRI], F32, tag="vg", name=f"vg{r}")
            nc.sync.dma_start(vg[:], bass.AP(
                v_dram, rowbase(r) * VF,
                [[8 * VF, K], [NHR * BD, NHO], [1, HRI]]))
            return vg

        def phase1_init():
            tu = tupool.tile([P, TUP], F32, tag="tu", name="tu0")
            ag0, vg0 = g_A(0), g_v(0)
            # T := A_0 ; ag block content is (hr, col, row)
            nc.vector.tensor_copy(
                bass.AP(tu.tensor, tu[:].offset,
                        [[TUP, P], [BD * JW, NHR], [JW, BD], [1, BD]]),
                bass.AP(ag0.tensor, ag0[:].offset,
                        [[HL * BD, P], [BD * BD, NHR], [1, BD], [BD, BD]]))
            nc.vector.tensor_copy(
                bass.AP(tu.tensor, tu[:].offset + BD,
                        [[TUP, P], [BD * JW, NHR], [JW, BD]]),
                bass.AP(vg0.tensor, vg0[:].offset,
                        [[HRI, P], [BD, NHR], [1, BD]]))
            tu_box['tu'] = tu

        def phase1_step(r):
            tu = tu_box['tu']
            ag, vg = g_A(r), g_v(r)
            mo = mopool.tile([P, TUP * BD], F32, tag="mo", name=f"mo{r}")
            for hr in range(NHR):
                # out[i, j9, k8] = A[i, k] * Tu[k, j]; A elem (i,k) at k*8+i
                nc.vector.tensor_tensor(
                    bass.AP(mo.tensor, mo[:].offset + hr * BD * JW * BD,
                            [[TUP * BD, P], [JW * BD, BD], [BD, JW], [1, BD]]),
                    bass.AP(ag.tensor, ag[:].offset + hr * BD * BD,
                            [[HL * BD, P], [1, BD], [0, JW], [BD, BD]]),
                    bass.AP(tu.tensor, tu[:].offset + hr * BD * JW,
                            [[TUP, P], [0, BD], [1, JW], [JW, BD]]),
                    ALU.mult)
            tun = tupool.tile([P, TUP], F32, tag="tu", name=f"tu{r}")
            nc.vector.tensor_reduce(
                bass.AP(tun.tensor, tun[:].offset, [[TUP, P], [1, TUP]]),
                bass.AP(mo.tensor, mo[:].offset,
                        [[TUP * BD, P], [BD, TUP], [1, BD]]),
                axis=mybir.AxisListType.X, op=ALU.add)
            nc.vector.tensor_tensor(
                bass.AP(tun.tensor, tun[:].offset + BD,
                        [[TUP, P], [BD * JW, NHR], [JW, BD]]),
                bass.AP(tun.tensor, tun[:].offset + BD,
                        [[TUP, P], [BD * JW, NHR], [JW, BD]]),
                bass.AP(vg.tensor, vg[:].offset,
                        [[HRI, P], [BD, NHR], [1, BD]]),
                ALU.add)
            tu_box['tu'] = tun

        # ================= stage A (+ interleaved phase 1) =================
        for q in range(NQ if nq_steps is None else nq_steps):
            RPQ = TPQ * 8  # r-range covered by this q
            xq = xpool.tile([P, 4, QT], BF16, tag="xq")
            for ttq in range(TPQ):
                # tile tau = q*TPQ+ttq: tokens c*C + 8*tau + j, col order (c, j)
                for k in range(4):
                    nc.sync.dma_start(
                        xq[:, k, bass.ts(ttq, P)],
                        bass.AP(xT, k * P * TOK + q * RPQ + ttq * 8,
                                [[TOK, P], [C, K], [1, 8]]))

            hid_t = hpool.tile([P, HID // P, QT], BF16, tag="hid")
            for m in range(HID // P):
                w1m = w2pool.tile([P, 4, P], BF16, tag="w1m", name=f"w1m{q}_{m}")
                nc.sync.dma_start(
                    w1m[:], w1[:, bass.ts(m, P)].rearrange("(k p) m -> p k m", p=P))
                ps = l1ps.tile([P, QT], F32, tag="l1")
                for k in range(4):
                    nc.tensor.matmul(ps[:], w1m[:, k, :], xq[:, k, :],
                                     start=(k == 0), stop=(k == 3))
                nc.scalar.activation(hid_t[:, m, :], ps[:], AF.Relu,
                                     bias=b1_s[:, m:m + 1])

            hv_t = hpool.tile([P, 4, QT], BF16, tag="hv")
            for m in range(4):
                ps = l1ps.tile([P, QT], F32, tag="l1")
                for k in range(4):
                    nc.tensor.matmul(ps[:], v1_s[:, k, bass.ts(m, P)], xq[:, k, :],
                                     start=(k == 0), stop=(k == 3))
                nc.scalar.activation(hv_t[:, m, :], ps[:], AF.Relu,
                                     bias=c1_s[:, m:m + 1])

            # ---- L2: token-major blk, W2 streamed per (n, k) ----
            blks = [blkpool.tile([P, NF], F32, tag="blk", name=f"blk{q}_{i}") for i in range(TPQ)]
            for n in range(NF // 512):
                pss = [l2ps.tile([P, 512], F32, tag="l2", name=f"l2ps{q}_{n}_{i}") for i in range(TPQ)]
                for ttq in range(TPQ):
                    nc.tensor.matmul(pss[ttq][:], ones_s[:1, :],
                                     b2_s[:1, bass.ts(n, 512)], start=True, stop=False)
                for k in range(HID // P):
                    w2s = w2pool.tile([P, 512], BF16, tag="w2s")
                    nc.sync.dma_start(w2s[:], w2[bass.ts(k, P), bass.ts(n, 512)])
                    for ttq in range(TPQ):
                        nc.tensor.matmul(pss[ttq][:], hid_t[:, k, bass.ts(ttq, P)],
                                         w2s[:], start=False, stop=(k == HID // P - 1))
                for ttq in range(TPQ):
                    nc.scalar.activation(blks[ttq][:, bass.ts(n, 512)], pss[ttq][:],
                                         AF.Identity)

            # ---- v2 + normalization per tok-tile ----
            for ttq in range(TPQ):
                tt = q * TPQ + ttq
                rowsl = bass.ds(tt * P, P)

                psv = vps.tile([P, VF], F32, tag="v")
                nc.tensor.matmul(psv[:], ones_s[:1, :], c2_s[:1, :],
                                 start=True, stop=False)
                for k in range(4):
                    nc.tensor.matmul(psv[:], hv_t[:, k, bass.ts(ttq, P)],
                                     v2_s[:, k, :], start=False, stop=(k == 3))
                vt = vtpool.tile([P, VF], F32, tag="vt")
                nc.scalar.activation(vt[:], psv[:], AF.Identity)
                nc.sync.dma_start(v_dram[rowsl, :], vt[:])

                blk = blks[ttq]
                pw = pwpool.tile([P, NF], F32, tag="pw")
                nc.scalar.activation(pw[:], blk[:], AF.Square)
                nc.scalar.activation(pw[:], pw[:], AF.Ln)
                nc.scalar.activation(pw[:], pw[:], AF.Exp, scale=0.6)
                # sum over i: feat = h*64 + i*8 + j -> dims [p, h, j, i]
                pst = smpool.tile([P, HL * BD], F32, tag="pst")
                nc.vector.tensor_reduce(
                    pst[:].rearrange("p (h j) -> p h j", h=HL, j=BD),
                    bass.AP(pw.tensor, pw[:].offset,
                            [[NF, P], [64, HL], [1, BD], [8, BD]]),
                    axis=mybir.AxisListType.X, op=ALU.add)
                nc.scalar.activation(pst[:], pst[:], AF.Ln)
                nc.scalar.activation(pst[:], pst[:], AF.Exp, scale=1.0 / 1.2)
                dm = smpool.tile([P, HL], F32, tag="dm")
                nc.vector.tensor_reduce(
                    dm[:].rearrange("p (h one) -> p h one", h=HL, one=1),
                    pst[:].rearrange("p (h j) -> p h j", h=HL, j=BD),
                    axis=mybir.AxisListType.X, op=ALU.max)
                rc = smpool.tile([P, HL], F32, tag="rc")
                nc.vector.reciprocal(rc[:], dm[:])
                # A = blk * rc (broadcast over i, j) -> into pw buffer
                # write A transposed per head: feat order (h, col j, row i)
                nc.vector.tensor_tensor(
                    bass.AP(pw.tensor, pw[:].offset,
                            [[NF, P], [64, HL], [1, BD], [8, BD]]),
                    blk[:].rearrange("p (h i j) -> p h i j", h=HL, i=BD, j=BD),
                    bass.AP(rc.tensor, rc[:].offset,
                            [[HL, P], [1, HL], [0, BD], [0, BD]]),
                    ALU.mult)
                nc.sync.dma_start(a_dram[rowsl, :], pw[:])

            # ---- phase 1 steps for this q's token tiles ----
            RPQ_ = TPQ * 8
            for r in range(q * RPQ_, (q + 1) * RPQ_):
                if p1_steps is not None and r >= p1_steps:
                    continue
                if r == 0:
                    phase1_init()
                else:
                    phase1_step(r)

        # ---- phase B: chunk-level combine (on partitions 0:NHO) ----
        tu = tu_box['tu']
        TUPK = K * TUP
        tu2 = scpool.tile([NHO, TUPK], F32, tag="tu2")
        for c in range(K):
            nc.sync.dma_start(tu2[:, c * TUP:(c + 1) * TUP],
                              tu[c * NHO:(c + 1) * NHO, :])
        s_seq = scpool.tile([NHO, (K + 1) * HRI], F32, tag="sseq")
        nc.vector.tensor_copy(s_seq[:, 0:HRI], a0_s[:])
        for c in range(K):
            mo3 = mopool.tile([NHO, HRI * BD], F32, tag="mo3")
            nc.vector.tensor_tensor(
                bass.AP(mo3.tensor, mo3[:].offset,
                        [[HRI * BD, NHO], [BD * BD, NHR], [BD, BD], [1, BD]]),
                bass.AP(tu2.tensor, tu2[:].offset + c * TUP,
                        [[TUPK, NHO], [BD * JW, NHR], [JW, BD], [1, BD]]),
                bass.AP(s_seq.tensor, s_seq[:].offset + c * HRI,
                        [[(K + 1) * HRI, NHO], [BD, NHR], [0, BD], [1, BD]]),
                ALU.mult)
            sn3 = smpool.tile([NHO, HRI], F32, tag="sn3")
            nc.vector.tensor_reduce(
                bass.AP(sn3.tensor, sn3[:].offset, [[HRI, NHO], [1, HRI]]),
                bass.AP(mo3.tensor, mo3[:].offset,
                        [[HRI * BD, NHO], [BD, HRI], [1, BD]]),
                axis=mybir.AxisListType.X, op=ALU.add)
            nc.vector.tensor_tensor(
                bass.AP(s_seq.tensor, s_seq[:].offset + (c + 1) * HRI,
                        [[(K + 1) * HRI, NHO], [BD, NHR], [1, BD]]),
                bass.AP(sn3.tensor, sn3[:].offset, [[HRI, NHO], [BD, NHR], [1, BD]]),
                bass.AP(tu2.tensor, tu2[:].offset + c * TUP + BD,
                        [[TUPK, NHO], [BD * JW, NHR], [JW, BD]]),
                ALU.add)
        # relayout chunk-start states -> s_init [(c,ho), (hr,i)]
        s_init = scpool.tile([P, HRI], F32, tag="sinit")
        for c in range(K):
            nc.sync.dma_start(s_init[c * NHO:(c + 1) * NHO, :],
                              s_seq[:, c * HRI:(c + 1) * HRI])

        # ---- phase C: re-run with true init ----
        def gather_A(r):
            ag = agpool.tile([P, HL * BD], F32, tag="agc", name=f"agc{r}")
            nc.sync.dma_start(ag[:], bass.AP(
                a_dram, rowbase(r) * NF,
                [[8 * NF, K], [NHR * BD * BD, NHO], [1, NHR * BD * BD]]))
            return ag

        def gather_v(r):
            vg = vgpool.tile([P, HRI], F32, tag="vgc", name=f"vgc{r}")
            nc.sync.dma_start(vg[:], bass.AP(
                v_dram, rowbase(r) * VF,
                [[8 * VF, K], [NHR * BD, NHO], [1, HRI]]))
            return vg

        s_out = scpool.tile([P, C * HRI], F32, tag="sout")
        for r in range(C if pc_steps is None else pc_steps):
            ag, vg = gather_A(r), gather_v(r)
            sprev = (bass.AP(s_init.tensor, s_init[:].offset,
                             [[HRI, P], [BD, NHR], [0, BD], [1, BD]])
                     if r == 0 else
                     bass.AP(s_out.tensor, s_out[:].offset + (r - 1) * HRI,
                             [[C * HRI, P], [BD, NHR], [0, BD], [1, BD]]))
            mo2 = mopool.tile([P, HRI * BD], F32, tag="mo2")
            nc.vector.tensor_tensor(
                bass.AP(mo2.tensor, mo2[:].offset,
                        [[HRI * BD, P], [BD * BD, NHR], [BD, BD], [1, BD]]),
                bass.AP(ag.tensor, ag[:].offset,
                        [[HL * BD, P], [BD * BD, NHR], [1, BD], [BD, BD]]),
                sprev, ALU.mult)
            sred = smpool.tile([P, HRI], F32, tag="sred")
            nc.vector.tensor_reduce(
                bass.AP(sred.tensor, sred[:].offset, [[HRI, P], [1, HRI]]),
                bass.AP(mo2.tensor, mo2[:].offset,
                        [[HRI * BD, P], [BD, HRI], [1, BD]]),
                axis=mybir.AxisListType.X, op=ALU.add)
            nc.vector.tensor_tensor(
                bass.AP(s_out.tensor, s_out[:].offset + r * HRI,
                        [[C * HRI, P], [1, HRI]]),
                bass.AP(sred.tensor, sred[:].offset, [[HRI, P], [1, HRI]]),
                bass.AP(vg.tensor, vg[:].offset, [[HRI, P], [1, HRI]]),
                ALU.add)

        # ---- output: s_out [(c,ho), (r, hr, i)] -> out [t, vf] ----
        for c in range(K):
            nc.sync.dma_start(
                bass.AP(out, c * C * VF, [[HRI, NHO], [VF, C], [1, HRI]]),
                bass.AP(s_out.tensor, s_out[c * NHO:(c + 1) * NHO, :].offset,
                        [[C * HRI, NHO], [HRI, C], [1, HRI]]))

    nc.compile()
    return nc


# ---------------- host side ----------------

_NC_CACHE = {}


def _get_nc(TOK=SEQ, K=16):
    key = (TOK, K)
    if key not in _NC_CACHE:
        _NC_CACHE[key] = build_nc(TOK=TOK, K=K)
    return _NC_CACHE[key]


def prep_shared(W1, b1, W2, b2, V1, c1, V2, c2, a0):
    bf = ml_dtypes.bfloat16
    W2r = W2.reshape(H, BD, BD, HID)
    W2c = (W2r - W2r.mean(axis=1, keepdims=True)).reshape(H * BD * BD, HID)
    b2r = b2.reshape(H, BD, BD)
    b2c = (b2r - b2r.mean(axis=1, keepdims=True)).reshape(-1)
    shared = {
        "smat": np.kron(np.eye(16, dtype=np.float32),
                        np.ones((BD, BD), np.float32)),
        "tinit": np.tile(np.concatenate([np.eye(BD, dtype=np.float32),
                                         np.zeros((BD, 1), np.float32)], 1)
                         .reshape(BD, 1, 9), (16, 16, 1)).reshape(128, -1),
        "w1": np.ascontiguousarray(W1.T).astype(bf),
        "b1": np.asarray(b1).reshape(HID, 1).astype(np.float32),
        "v1": np.ascontiguousarray(V1.T).astype(bf),
        "c1": np.asarray(c1).reshape(EMB, 1).astype(np.float32),
    }
    halves = []
    for half in range(2):
        rsl = slice(half * NF, (half + 1) * NF)
        vsl = slice(half * VF, (half + 1) * VF)
        hsl = slice(half * HL, (half + 1) * HL)
        a0h = np.asarray(a0)[0, hsl]                       # [32, 8]
        a0p = a0h.reshape(BD, 4, BD).reshape(BD, 32)       # [ho, (hr, i)]
        halves.append({
            "w2": np.ascontiguousarray(W2c[rsl].T).astype(bf),
            "b2": b2c[rsl].reshape(1, NF).astype(bf),
            "v2": np.ascontiguousarray(V2[vsl].T).astype(bf),
            "c2": np.asarray(c2)[vsl].reshape(1, VF).astype(bf),
            "a0": a0p.astype(np.float32),
        })
    return shared, halves


def make_in_maps(x, W1, b1, W2, b2, V1, c1, V2, c2, a0):
    shared, halves = prep_shared(W1, b1, W2, b2, V1, c1, V2, c2, a0)
    bf = ml_dtypes.bfloat16
    in_maps = []
    for core in range(N_CORES):
        b, half = core // 2, core % 2
        m = dict(shared)
        m.update(halves[half])
        m["xT"] = np.ascontiguousarray(np.asarray(x)[b].T).astype(bf)
        in_maps.append(m)
    return in_maps


def kernel(x, W1, b1, W2, b2, V1, c1, V2, c2, a0):
    from concourse import bass_utils
    nc = _get_nc(SEQ)
    in_maps = make_in_maps(x, W1, b1, W2, b2, V1, c1, V2, c2, a0)
    res = bass_utils.run_bass_kernel_spmd(nc, in_maps, core_ids=list(range(N_CORES)))
    out = np.zeros((BS, SEQ, EMB), np.float32)
    for core in range(N_CORES):
        b, half = core // 2, core % 2
        out[b, :, half * VF:(half + 1) * VF] = res.results[core]["out"]
    return out



# revision 2
# speedup vs baseline: 1.5628x; 1.5628x over previous
"""Trainium2 Bass kernel v2 for nn_BlockModel_82678120448388.

Per core (batch b, head-half): MLPs on PE (bf16, W1 resident, batched W2
stream), p=1.2 norm per 512-feat slice (Act + Pool reduce, one act table),
chunked scan (K=16 chunks x 128 steps) on DVE in fp16 with 2x_1p layouts:
state ST[jcol,i] = [T|u]^T updated per step via mult + k-tree adds.
Per-step [T|u] stored to DRAM (fp16); phase C is bulk-parallel
s_t = u_t + T_t sigma_c (split DVE/Pool, no serial tail).
"""

import numpy as np
import ml_dtypes
from contextlib import ExitStack

import concourse.bass as bass
import concourse.bacc as bacc
import concourse.tile as tile
from concourse import mybir

F32 = mybir.dt.float32
F16 = mybir.dt.float16
BF16 = mybir.dt.bfloat16
AF = mybir.ActivationFunctionType
ALU = mybir.AluOpType

BS, SEQ, EMB, BD = 4, 2048, 512, 8
H = EMB // BD
HL = 32            # heads per core
NF = HL * BD * BD  # 2048 blk feats per core
VF = HL * BD       # 256 v feats per core
HID = EMB * BD     # 4096
P = 128
K = 16             # chunks
NHO = P // K       # 8 head-groups on partitions
NHR = HL // NHO    # 4 heads per group (free dim)
JW = BD + 1        # 9 jcols of ST ([T|u]^T)
STW = NHR * JW * BD  # 288 = per-partition ST row
NT = SEQ // P      # 16 token tiles
QT = 512
NQ = SEQ // QT     # 4
TPQ = QT // P      # 4
NSL = 4            # 512-wide feat slices
SL = NF // NSL     # 512
HSL = HL // NSL    # 8 heads per slice
CHK = SEQ // K     # 128 positions per chunk

N_CORES = 8
ACT_SET = 6        # natural_log_exp_and_others: ln/exp/relu/square/copy

# tuning knobs
MULT_SPLIT_HR = False   # True: 4 smaller 4-dim mults instead of one 5-dim
PC_POOL_MOD = 4         # phase-C tiles with tau % PC_POOL_MOD == 3 go to Pool


def build_nc():
    nc = bacc.Bacc("TRN2", target_bir_lowering=False, debug=False)

    xT = nc.dram_tensor("xT", [EMB, SEQ], BF16, kind="ExternalInput")
    w1 = nc.dram_tensor("w1", [EMB, HID], BF16, kind="ExternalInput")
    b1 = nc.dram_tensor("b1", [HID, 1], F32, kind="ExternalInput")
    w2 = nc.dram_tensor("w2", [HID, NF], BF16, kind="ExternalInput")
    b2 = nc.dram_tensor("b2", [1, NF], BF16, kind="ExternalInput")
    v1 = nc.dram_tensor("v1", [EMB, EMB], BF16, kind="ExternalInput")
    c1 = nc.dram_tensor("c1", [EMB, 1], F32, kind="ExternalInput")
    v2 = nc.dram_tensor("v2", [EMB, VF], BF16, kind="ExternalInput")
    c2 = nc.dram_tensor("c2", [1, VF], BF16, kind="ExternalInput")
    a0 = nc.dram_tensor("a0", [NHO, NHR * BD], F16, kind="ExternalInput")
    out = nc.dram_tensor("out", [SEQ, VF], F32, kind="ExternalOutput")

    # scratch in scan order: [tau][c][ho][jr][payload]
    a_dram = nc.dram_tensor("a_scratch", [NT * P * NF], F16)
    v_dram = nc.dram_tensor("v_scratch", [NT * P * VF], F16)
    tu_dram = nc.dram_tensor("tu_scratch", [NT * P * BD * STW], F16)

    with ExitStack() as ctx:
        tc = ctx.enter_context(tile.TileContext(nc))
        cpool = ctx.enter_context(tc.tile_pool(name="consts", bufs=1))
        wpool = ctx.enter_context(tc.tile_pool(name="weights", bufs=1))
        xpool = ctx.enter_context(tc.tile_pool(name="xstream", bufs=2))
        hpool = ctx.enter_context(tc.tile_pool(name="hidden", bufs=1))
        hvpool = ctx.enter_context(tc.tile_pool(name="hv", bufs=1))
        w2pool = ctx.enter_context(tc.tile_pool(name="w2stream", bufs=3))
        l1ps = ctx.enter_context(tc.tile_pool(name="l1ps", bufs=2, space="PSUM"))
        l2ps = ctx.enter_context(tc.tile_pool(name="l2ps", bufs=TPQ, space="PSUM"))
        vps = ctx.enter_context(tc.tile_pool(name="vps", bufs=1, space="PSUM"))
        blkpool = ctx.enter_context(tc.tile_pool(name="blk", bufs=4))
        pwpool = ctx.enter_context(tc.tile_pool(name="pw", bufs=2))
        smpool = ctx.enter_context(tc.tile_pool(name="small", bufs=4))
        aipool = ctx.enter_context(tc.tile_pool(name="ai", bufs=3))
        vtpool = ctx.enter_context(tc.tile_pool(name="vtile", bufs=2))
        agpool = ctx.enter_context(tc.tile_pool(name="ag", bufs=2))
        vgpool = ctx.enter_context(tc.tile_pool(name="vg", bufs=2))
        tupool = ctx.enter_context(tc.tile_pool(name="tu", bufs=2))
        mopool = ctx.enter_context(tc.tile_pool(name="mo", bufs=2))
        scpool = ctx.enter_context(tc.tile_pool(name="scan", bufs=1))
        tcpool = ctx.enter_context(tc.tile_pool(name="tuc", bufs=3))
        tcpoolB = ctx.enter_context(tc.tile_pool(name="tucB", bufs=1))
        pcpoolB = ctx.enter_context(tc.tile_pool(name="pcB", bufs=1))

        nc.scalar.add_instruction(mybir.InstLoadActFuncSet(
            name=nc.get_next_instruction_name(), act_func_set_id=ACT_SET,
            ins=[], outs=[]))

        # ---- constants / weights ----
        ones_s = cpool.tile([1, P], BF16, tag="ones")
        nc.vector.memset(ones_s[:], 1.0)
        b1_s = cpool.tile([P, HID // P], F32, tag="b1")
        nc.sync.dma_start(b1_s[:], b1[:].rearrange("(m p) one -> p (m one)", p=P))
        c1_s = cpool.tile([P, EMB // P], F32, tag="c1")
        nc.sync.dma_start(c1_s[:], c1[:].rearrange("(m p) one -> p (m one)", p=P))
        b2_s = cpool.tile([1, NF], BF16, tag="b2")
        nc.sync.dma_start(b2_s[:], b2[:])
        c2_s = cpool.tile([1, VF], BF16, tag="c2")
        nc.sync.dma_start(c2_s[:], c2[:])
        a0_s = cpool.tile([NHO, NHR * BD], F16, tag="a0")
        nc.sync.dma_start(a0_s[:], a0[:])

        xq0 = xpool.tile([P, 4, QT], BF16, tag="xq", name="xq_pre0")
        for ti in range(4):
            for k in range(4):
                nc.sync.dma_start(
                    xq0[:, k, bass.ts(ti, P)],
                    bass.AP(xT, k * P * SEQ + ti * 8,
                            [[SEQ, P], [CHK, K], [1, 8]]))

        w1_s = wpool.tile([P, 4, HID], BF16, tag="w1")
        for k in range(4):
            for mg in range(4):
                nc.sync.dma_start(
                    w1_s[:, k, bass.ds(mg * HID // 4, HID // 4)],
                    bass.AP(w1, k * P * HID + mg * HID // 4,
                            [[HID, P], [1, HID // 4]]))
        v1_s = wpool.tile([P, 4, EMB], BF16, tag="v1")
        nc.sync.dma_start(v1_s[:], v1[:].rearrange("(k p) m -> p k m", p=P))
        v2_s = wpool.tile([P, 4, VF], BF16, tag="v2")
        nc.sync.dma_start(v2_s[:], v2[:].rearrange("(k p) n -> p k n", p=P))

        # ================= phase-1 =================
        # per partition (c,ho), per hr: ST[jcol, i] fp16 (jcol 9, i 8)
        # step: ST_new[jcol,i] = sum_k ST[jcol,k]*A[i,k]; u row += v
        tu_box = {}

        def p1_step(tau, jr, a8, v8, tu8):
            if tau == 0 and jr == 0:
                nc.vector.tensor_copy(
                    bass.AP(tu8.tensor, tu8[:].offset,
                            [[BD * STW, P], [JW * BD, NHR], [1, BD], [BD, BD]]),
                    bass.AP(a8.tensor, a8[:].offset,
                            [[BD * NHR * BD * BD, P], [BD * BD, NHR],
                             [BD, BD], [1, BD]]))
                nc.vector.tensor_copy(
                    bass.AP(tu8.tensor, tu8[:].offset + BD * BD,
                            [[BD * STW, P], [JW * BD, NHR], [1, BD]]),
                    bass.AP(v8.tensor, v8[:].offset,
                            [[BD * NHR * BD, P], [BD, NHR], [1, BD]]))
                return
            if jr == 0:
                src_t, src_off = tu_box['prev'], (BD - 1) * STW
            else:
                src_t, src_off = tu8, (jr - 1) * STW
            mo = mopool.tile([P, NHR * JW * BD * BD], F16, tag="mo",
                             name=f"mo{tau}_{jr}")
            # mo[hr, jcol, i, k] = ST[hr, jcol, k] * A[hr, i, k]
            if MULT_SPLIT_HR:
                for hr in range(NHR):
                    nc.vector.tensor_tensor(
                        bass.AP(mo.tensor, mo[:].offset + hr * JW * BD * BD,
                                [[NHR * JW * BD * BD, P], [BD * BD, JW],
                                 [BD, BD], [1, BD]]),
                        bass.AP(src_t.tensor,
                                src_t[:].offset + src_off + hr * JW * BD,
                                [[BD * STW, P], [BD, JW], [0, BD], [1, BD]]),
                        bass.AP(a8.tensor,
                                a8[:].offset + jr * NHR * BD * BD + hr * BD * BD,
                                [[BD * NHR * BD * BD, P], [0, JW],
                                 [BD, BD], [1, BD]]),
                        ALU.mult)
            else:
                nc.vector.tensor_tensor(
                    bass.AP(mo.tensor, mo[:].offset,
                            [[NHR * JW * BD * BD, P], [JW * BD * BD, NHR],
                             [BD * BD, JW], [BD, BD], [1, BD]]),
                    bass.AP(src_t.tensor, src_t[:].offset + src_off,
                            [[BD * STW, P], [JW * BD, NHR],
                             [BD, JW], [0, BD], [1, BD]]),
                    bass.AP(a8.tensor, a8[:].offset + jr * NHR * BD * BD,
                            [[BD * NHR * BD * BD, P], [BD * BD, NHR],
                             [0, JW], [BD, BD], [1, BD]]),
                    ALU.mult)
            m1 = mopool.tile([P, NHR * JW * BD * 4], F16, tag="m1",
                             name=f"m1{tau}_{jr}")
            nc.vector.tensor_tensor(
                m1[:],
                bass.AP(mo.tensor, mo[:].offset,
                        [[NHR * JW * BD * BD, P], [BD, NHR * JW * BD], [1, 4]]),
                bass.AP(mo.tensor, mo[:].offset + 4,
                        [[NHR * JW * BD * BD, P], [BD, NHR * JW * BD], [1, 4]]),
                ALU.add)
            m2 = mopool.tile([P, NHR * JW * BD * 2], F16, tag="m2",
                             name=f"m2{tau}_{jr}")
            nc.vector.tensor_tensor(
                m2[:],
                bass.AP(m1.tensor, m1[:].offset,
                        [[NHR * JW * BD * 4, P], [4, NHR * JW * BD], [1, 2]]),
                bass.AP(m1.tensor, m1[:].offset + 2,
                        [[NHR * JW * BD * 4, P], [4, NHR * JW * BD], [1, 2]]),
                ALU.add)
            nc.vector.tensor_tensor(
                bass.AP(tu8.tensor, tu8[:].offset + jr * STW,
                        [[BD * STW, P], [1, STW]]),
                bass.AP(m2.tensor, m2[:].offset,
                        [[NHR * JW * BD * 2, P], [2, STW]]),
                bass.AP(m2.tensor, m2[:].offset + 1,
                        [[NHR * JW * BD * 2, P], [2, STW]]),
                ALU.add)
            urow = bass.AP(tu8.tensor, tu8[:].offset + jr * STW + BD * BD,
                           [[BD * STW, P], [JW * BD, NHR], [1, BD]])
            nc.vector.tensor_tensor(
                urow, urow,
                bass.AP(v8.tensor, v8[:].offset + jr * NHR * BD,
                        [[BD * NHR * BD, P], [BD, NHR], [1, BD]]),
                ALU.add)

        def p1_tile(tau):
            a8 = agpool.tile([P, BD, NHR * BD * BD], F16, tag="a8",
                             name=f"a8_{tau}")
            v8 = vgpool.tile([P, BD, NHR * BD], F16, tag="v8", name=f"v8_{tau}")
            nc.gpsimd.dma_start(a8[:], bass.AP(
                a_dram, tau * P * NF, [[BD * 256, P], [1, BD * 256]]))
            nc.gpsimd.dma_start(v8[:], bass.AP(
                v_dram, tau * P * VF, [[BD * 32, P], [1, BD * 32]]))
            tu8 = tupool.tile([P, BD, STW], F16, tag="tu8", name=f"tu8_{tau}")
            for jr in range(BD):
                p1_step(tau, jr, a8, v8, tu8)
            tu_box['prev'] = tu8
            nc.scalar.dma_start(
                bass.AP(tu_dram, tau * P * BD * STW,
                        [[BD * STW, P], [1, BD * STW]]),
                tu8[:])

        # ================= stage A =================
        SCHED = [4, 4, 4, 2, 2]  # tiles per q-group (tapered tail)
        t0 = 0
        for qi, nt_q in enumerate(SCHED):
            W = nt_q * P
            if qi == 0:
                xq = xq0
            else:
                xq = xpool.tile([P, 4, QT], BF16, tag="xq", name=f"xq{qi}")
                for ti in range(nt_q):
                    for k in range(4):
                        nc.sync.dma_start(
                            xq[:, k, bass.ts(ti, P)],
                            bass.AP(xT, k * P * SEQ + (t0 + ti) * 8,
                                    [[SEQ, P], [CHK, K], [1, 8]]))

            hid_t = hpool.tile([P, HID // P, QT], BF16, tag="hid",
                               name=f"hid{qi}")
            for m in range(HID // P):
                ps = l1ps.tile([P, QT], F32, tag="l1")
                for k in range(4):
                    nc.tensor.matmul(ps[:, :W], w1_s[:, k, bass.ts(m, P)],
                                     xq[:, k, :W], start=(k == 0),
                                     stop=(k == 3))
                nc.scalar.activation(hid_t[:, m, :W], ps[:, :W], AF.Relu,
                                     bias=b1_s[:, m:m + 1])

            hv_t = hvpool.tile([P, 4, QT], BF16, tag="hv", name=f"hv{qi}")
            for m in range(4):
                ps = l1ps.tile([P, QT], F32, tag="l1")
                for k in range(4):
                    nc.tensor.matmul(ps[:, :W], v1_s[:, k, bass.ts(m, P)],
                                     xq[:, k, :W], start=(k == 0),
                                     stop=(k == 3))
                nc.scalar.activation(hv_t[:, m, :W], ps[:, :W], AF.Relu,
                                     bias=c1_s[:, m:m + 1])

            # ---- L2 + per-slice norm ----
            for n in range(NSL):
                pss = [l2ps.tile([P, SL], F32, tag="l2",
                                 name=f"l2_{qi}_{n}_{i}")
                       for i in range(nt_q)]
                for ti in range(nt_q):
                    nc.tensor.matmul(pss[ti][:], ones_s[:1, :],
                                     b2_s[:1, bass.ts(n, SL)],
                                     start=True, stop=False)
                for kg in range(8):
                    w2s = w2pool.tile([P, 4, SL], BF16, tag="w2s",
                                      name=f"w2_{qi}_{n}_{kg}")
                    nc.sync.dma_start(w2s[:], bass.AP(
                        w2, kg * 4 * P * NF + n * SL,
                        [[NF, P], [P * NF, 4], [1, SL]]))
                    for k8 in range(4):
                        k = kg * 4 + k8
                        for ti in range(nt_q):
                            nc.tensor.matmul(
                                pss[ti][:], hid_t[:, k, bass.ts(ti, P)],
                                w2s[:, k8, :], start=False,
                                stop=(k == HID // P - 1))
                # psum-freeing copies first, then the norm chains
                blks = []
                for ti in range(nt_q):
                    blk = blkpool.tile([P, SL], F32, tag="blk",
                                       name=f"blk{qi}_{n}_{ti}")
                    nc.scalar.activation(blk[:], pss[ti][:], AF.Identity)
                    blks.append(blk)
                for ti in range(nt_q):
                    blk = blks[ti]
                    pw = pwpool.tile([P, SL], F32, tag="pw")
                    nc.scalar.activation(pw[:], blk[:], AF.Square)
                    nc.scalar.activation(pw[:], pw[:], AF.Ln)
                    nc.scalar.activation(pw[:], pw[:], AF.Exp, scale=0.6)
                    pst = smpool.tile([P, HSL * BD], F32, tag="pst")
                    nc.vector.tensor_reduce(
                        pst[:].rearrange("p (h j) -> p h j", h=HSL, j=BD),
                        bass.AP(pw.tensor, pw[:].offset,
                                [[SL, P], [64, HSL], [1, BD], [8, BD]]),
                        axis=mybir.AxisListType.X, op=ALU.add)
                    dm = smpool.tile([P, HSL], F32, tag="dm")
                    nc.vector.tensor_reduce(
                        dm[:].rearrange("p (h one) -> p h one", h=HSL, one=1),
                        pst[:].rearrange("p (h j) -> p h j", h=HSL, j=BD),
                        axis=mybir.AxisListType.X, op=ALU.max)
                    rc = smpool.tile([P, HSL], F32, tag="rc")
                    nc.scalar.activation(rc[:], dm[:], AF.Ln)
                    nc.scalar.activation(rc[:], rc[:], AF.Exp,
                                         scale=-1.0 / 1.2)
                    asl = aipool.tile([P, SL], F16, tag="ai",
                                      name=f"ai{qi}_{n}_{ti}")
                    nc.gpsimd.tensor_tensor(
                        asl[:].rearrange("p (h i k) -> p h i k",
                                         h=HSL, i=BD, k=BD),
                        blk[:].rearrange("p (h i k) -> p h i k",
                                         h=HSL, i=BD, k=BD),
                        bass.AP(rc.tensor, rc[:].offset,
                                [[HSL, P], [1, HSL], [0, BD], [0, BD]]),
                        ALU.mult)
                    tt = t0 + ti
                    for ho2 in range(2):
                        # dst (tau, c, ho=2n+ho2, jr=j, (hr,i,k))
                        nc.gpsimd.dma_start(
                            bass.AP(a_dram,
                                    tt * P * NF + (2 * n + ho2) * BD * 256,
                                    [[NHO * BD * 256, K], [256, BD],
                                     [1, 256]]),
                            bass.AP(asl.tensor, asl[:].offset + ho2 * 256,
                                    [[SL, P], [1, 256]]))

            for ti in range(nt_q):
                tt = t0 + ti
                psv = vps.tile([P, VF], F32, tag="v")
                nc.tensor.matmul(psv[:], ones_s[:1, :], c2_s[:1, :],
                                 start=True, stop=False)
                for k in range(4):
                    nc.tensor.matmul(psv[:], hv_t[:, k, bass.ts(ti, P)],
                                     v2_s[:, k, :], start=False,
                                     stop=(k == 3))
                vt = vtpool.tile([P, VF], F16, tag="vt")
                nc.scalar.activation(vt[:], psv[:], AF.Identity)
                for ho in range(NHO):
                    nc.gpsimd.dma_start(
                        bass.AP(v_dram, tt * P * VF + ho * BD * 32,
                                [[NHO * BD * 32, K], [32, BD], [1, 32]]),
                        bass.AP(vt.tensor, vt[:].offset + ho * 32,
                                [[VF, P], [1, 32]]))

                p1_tile(tt)
            t0 += nt_q

        # ================= phase B =================
        # extract per-chunk summaries from tu8 of the last tile (SBUF->SBUF)
        tu_last = tu_box['prev']
        tub = scpool.tile([NHO, K * STW], F16, tag="tub")
        for c in range(K):
            qeng = nc.sync if c % 2 == 0 else nc.scalar
            qeng.dma_start(
                bass.AP(tub.tensor, tub[:].offset + c * STW,
                        [[K * STW, NHO], [1, STW]]),
                bass.AP(tu_last.tensor,
                        tu_last[:].offset + c * NHO * BD * STW
                        + (BD - 1) * STW,
                        [[BD * STW, NHO], [1, STW]]))
        sg = scpool.tile([NHO, (K + 1) * NHR * BD], F16, tag="sg")
        nc.vector.tensor_copy(sg[:, 0:NHR * BD], a0_s[:])
        for c in range(K):
            mB = smpool.tile([NHO, NHR * BD * BD], F32, tag="mB", name=f"mB{c}")
            nc.vector.tensor_tensor(
                bass.AP(mB.tensor, mB[:].offset,
                        [[NHR * BD * BD, NHO], [BD * BD, NHR],
                         [BD, BD], [1, BD]]),
                bass.AP(tub.tensor, tub[:].offset + c * STW,
                        [[K * STW, NHO], [JW * BD, NHR], [BD, BD], [1, BD]]),
                bass.AP(sg.tensor, sg[:].offset + c * NHR * BD,
                        [[(K + 1) * NHR * BD, NHO], [BD, NHR],
                         [1, BD], [0, BD]]),
                ALU.mult)
            sn = smpool.tile([NHO, NHR * BD], F32, tag="sn", name=f"sn{c}")
            nc.vector.tensor_reduce(
                bass.AP(sn.tensor, sn[:].offset,
                        [[NHR * BD, NHO], [BD, NHR], [1, BD]]),
                bass.AP(mB.tensor, mB[:].offset,
                        [[NHR * BD * BD, NHO], [BD * BD, NHR],
                         [1, BD], [BD, BD]]),
                axis=mybir.AxisListType.X, op=ALU.add)
            nc.vector.tensor_tensor(
                bass.AP(sg.tensor, sg[:].offset + (c + 1) * NHR * BD,
                        [[(K + 1) * NHR * BD, NHO], [1, NHR * BD]]),
                bass.AP(sn.tensor, sn[:].offset,
                        [[NHR * BD, NHO], [1, NHR * BD]]),
                bass.AP(tub.tensor, tub[:].offset + c * STW + BD * BD,
                        [[K * STW, NHO], [JW * BD, NHR], [1, BD]]),
                ALU.add)

        # scatter sigma_c -> sc[(c,ho), (hr,k)], then expand over i
        sc_t = scpool.tile([P, NHR * BD], F16, tag="sc")
        for ho in range(NHO):
            nc.sync.dma_start(
                sc_t[ho::NHO, :],
                bass.AP(sg.tensor, sg[:].offset + ho * (K + 1) * NHR * BD,
                        [[(K + 1) * NHR * BD, 1], [NHR * BD, K],
                         [1, NHR * BD]]))
        sexp = scpool.tile([P, NHR * BD * BD], F16, tag="sexp")
        nc.vector.tensor_copy(
            bass.AP(sexp.tensor, sexp[:].offset,
                    [[NHR * BD * BD, P], [BD, NHR * BD], [1, BD]]),
            bass.AP(sc_t.tensor, sc_t[:].offset,
                    [[NHR * BD, P], [1, NHR * BD], [0, BD]]))

        # ================= phase C =================
        NTD = NT - 4  # DVE handles tau 0..11, Pool 12..15
        s_outA = scpool.tile([P, NTD * BD * NHR * BD], F32, tag="soutA")
        s_outB = scpool.tile([P, 4 * BD * NHR * BD], F32, tag="soutB")
        PCW = BD * NHR * BD * BD  # 2048
        for tau in range(NT):
            on_pool = (tau >= NTD)
            eng = nc.gpsimd if on_pool else nc.vector
            tpool = tcpoolB if on_pool else tcpool
            tuc = tpool.tile([P, BD, STW], F16, tag="tuc", name=f"tuc{tau}")
            nc.scalar.dma_start(tuc[:], bass.AP(
                tu_dram, tau * P * BD * STW,
                [[BD * STW, P], [1, BD * STW]]))
            if on_pool:
                m = pcpoolB.tile([P, PCW], F16, tag="pcm", name=f"pcm{tau}")
                m1 = pcpoolB.tile([P, PCW // 2], F16, tag="pcm1",
                                  name=f"pcm1{tau}")
                m2 = pcpoolB.tile([P, PCW // 4], F16, tag="pcm2",
                                  name=f"pcm2{tau}")
                m3 = pcpoolB.tile([P, PCW // 8], F16, tag="pcm3",
                                  name=f"pcm3{tau}")
            else:
                m = mopool.tile([P, NHR * JW * BD * BD], F16, tag="mo",
                                name=f"pcm{tau}")
                m1 = mopool.tile([P, NHR * JW * BD * 4], F16, tag="m1",
                                 name=f"pcm1{tau}")
                m2 = mopool.tile([P, NHR * JW * BD * 2], F16, tag="m2",
                                 name=f"pcm2{tau}")
                m3 = mopool.tile([P, NHR * JW * BD * BD], F16, tag="mo",
                                 name=f"pcm3{tau}")
            MP = m[:].ap[0][0]
            M1P = m1[:].ap[0][0]
            M2P = m2[:].ap[0][0]
            M3P = m3[:].ap[0][0]
            # m[jr, hr, (k,i)] = ST[jr,hr,jcol=k,i] * sigma_exp[hr,(k,i)]
            eng.tensor_tensor(
                bass.AP(m.tensor, m[:].offset,
                        [[MP, P], [NHR * BD * BD, BD],
                         [BD * BD, NHR], [1, BD * BD]]),
                bass.AP(tuc.tensor, tuc[:].offset,
                        [[BD * STW, P], [STW, BD], [JW * BD, NHR],
                         [1, BD * BD]]),
                bass.AP(sexp.tensor, sexp[:].offset,
                        [[NHR * BD * BD, P], [0, BD], [BD * BD, NHR],
                         [1, BD * BD]]),
                ALU.mult)
            eng.tensor_tensor(
                bass.AP(m1.tensor, m1[:].offset, [[M1P, P], [1, PCW // 2]]),
                bass.AP(m.tensor, m[:].offset,
                        [[MP, P], [BD * BD, BD * NHR], [1, 4 * BD]]),
                bass.AP(m.tensor, m[:].offset + 4 * BD,
                        [[MP, P], [BD * BD, BD * NHR], [1, 4 * BD]]),
                ALU.add)
            eng.tensor_tensor(
                bass.AP(m2.tensor, m2[:].offset, [[M2P, P], [1, PCW // 4]]),
                bass.AP(m1.tensor, m1[:].offset,
                        [[M1P, P], [4 * BD, BD * NHR], [1, 2 * BD]]),
                bass.AP(m1.tensor, m1[:].offset + 2 * BD,
                        [[M1P, P], [4 * BD, BD * NHR], [1, 2 * BD]]),
                ALU.add)
            eng.tensor_tensor(
                bass.AP(m3.tensor, m3[:].offset, [[M3P, P], [1, PCW // 8]]),
                bass.AP(m2.tensor, m2[:].offset,
                        [[M2P, P], [2 * BD, BD * NHR], [1, BD]]),
                bass.AP(m2.tensor, m2[:].offset + BD,
                        [[M2P, P], [2 * BD, BD * NHR], [1, BD]]),
                ALU.add)
            # s = m3 + u  -> s_out half (f32)
            s_o = s_outB if on_pool else s_outA
            s_off = (tau - NTD if on_pool else tau) * BD * NHR * BD
            eng.tensor_tensor(
                bass.AP(s_o.tensor, s_o[:].offset + s_off,
                        [[s_o[:].ap[0][0], P], [1, BD * NHR * BD]]),
                bass.AP(m3.tensor, m3[:].offset, [[M3P, P], [1, PCW // 8]]),
                bass.AP(tuc.tensor, tuc[:].offset + BD * BD,
                        [[BD * STW, P], [STW, BD], [JW * BD, NHR], [1, BD]]),
                ALU.add)

        # ---- output: s_out halves [(c,ho), (pos, hr, i)] -> out[t, vf] ----
        for c in range(K):
            nc.sync.dma_start(
                bass.AP(out, c * CHK * VF,
                        [[NHR * BD, NHO], [VF, NTD * BD], [1, NHR * BD]]),
                bass.AP(s_outA.tensor,
                        s_outA[c * NHO:(c + 1) * NHO, :].offset,
                        [[NTD * BD * NHR * BD, NHO], [NHR * BD, NTD * BD],
                         [1, NHR * BD]]))
            nc.scalar.dma_start(
                bass.AP(out, (c * CHK + NTD * BD) * VF,
                        [[NHR * BD, NHO], [VF, 4 * BD], [1, NHR * BD]]),
                bass.AP(s_outB.tensor,
                        s_outB[c * NHO:(c + 1) * NHO, :].offset,
                        [[4 * BD * NHR * BD, NHO], [NHR * BD, 4 * BD],
                         [1, NHR * BD]]))

    nc.compile()
    return nc


# ---------------- host side ----------------

_NC_CACHE = {}


def _get_nc(TOK=SEQ):
    if TOK not in _NC_CACHE:
        _NC_CACHE[TOK] = build_nc()
    return _NC_CACHE[TOK]


def prep_shared(W1, b1, W2, b2, V1, c1, V2, c2, a0):
    bf = ml_dtypes.bfloat16
    W2r = np.asarray(W2).reshape(H, BD, BD, HID)
    W2c = (W2r - W2r.mean(axis=1, keepdims=True)).reshape(H * BD * BD, HID)
    b2r = np.asarray(b2).reshape(H, BD, BD)
    b2c = (b2r - b2r.mean(axis=1, keepdims=True)).reshape(-1)
    shared = {
        "w1": np.ascontiguousarray(np.asarray(W1).T).astype(bf),
        "b1": np.asarray(b1).reshape(HID, 1).astype(np.float32),
        "v1": np.ascontiguousarray(np.asarray(V1).T).astype(bf),
        "c1": np.asarray(c1).reshape(EMB, 1).astype(np.float32),
    }
    halves = []
    for half in range(2):
        rsl = slice(half * NF, (half + 1) * NF)
        vsl = slice(half * VF, (half + 1) * VF)
        hsl = slice(half * HL, (half + 1) * HL)
        a0h = np.asarray(a0)[0, hsl]                       # [32, 8]
        a0p = a0h.reshape(NHO, NHR * BD)                   # [8, 32]
        halves.append({
            "w2": np.ascontiguousarray(W2c[rsl].T).astype(bf),
            "b2": b2c[rsl].reshape(1, NF).astype(bf),
            "v2": np.ascontiguousarray(np.asarray(V2)[vsl].T).astype(bf),
            "c2": np.asarray(c2)[vsl].reshape(1, VF).astype(bf),
            "a0": a0p.astype(np.float16),
        })
    return shared, halves


def make_in_maps(x, W1, b1, W2, b2, V1, c1, V2, c2, a0):
    shared, halves = prep_shared(W1, b1, W2, b2, V1, c1, V2, c2, a0)
    bf = ml_dtypes.bfloat16
    in_maps = []
    for core in range(N_CORES):
        b, half = core // 2, core % 2
        m = dict(shared)
        m.update(halves[half])
        m["xT"] = np.ascontiguousarray(np.asarray(x)[b].T).astype(bf)
        in_maps.append(m)
    return in_maps


def kernel(x, W1, b1, W2, b2, V1, c1, V2, c2, a0):
    from concourse import bass_utils
    nc = _get_nc(SEQ)
    in_maps = make_in_maps(x, W1, b1, W2, b2, V1, c1, V2, c2, a0)
    res = bass_utils.run_bass_kernel_spmd(nc, in_maps,
                                          core_ids=list(range(N_CORES)))
    out = np.zeros((BS, SEQ, EMB), np.float32)
    for core in range(N_CORES):
        b, half = core // 2, core % 2
        out[b, :, half * VF:(half + 1) * VF] = res.results[core]["out"]
    return out


# revision 3
# speedup vs baseline: 1.5730x; 1.0065x over previous
"""Trainium2 Bass kernel v2 for nn_BlockModel_82678120448388.

Per core (batch b, head-half): MLPs on PE (bf16, W1 resident, batched W2
stream), p=1.2 norm per 512-feat slice (Act + Pool reduce, one act table),
chunked scan (K=16 chunks x 128 steps) on DVE in fp16 with 2x_1p layouts:
state ST[jcol,i] = [T|u]^T updated per step via mult + k-tree adds.
Per-step [T|u] stored to DRAM (fp16); phase C is bulk-parallel
s_t = u_t + T_t sigma_c (split DVE/Pool, no serial tail).
"""

import numpy as np
import ml_dtypes
from contextlib import ExitStack

import concourse.bass as bass
import concourse.bacc as bacc
import concourse.tile as tile
from concourse import mybir

F32 = mybir.dt.float32
F16 = mybir.dt.float16
BF16 = mybir.dt.bfloat16
AF = mybir.ActivationFunctionType
ALU = mybir.AluOpType

BS, SEQ, EMB, BD = 4, 2048, 512, 8
H = EMB // BD
HL = 32            # heads per core
NF = HL * BD * BD  # 2048 blk feats per core
VF = HL * BD       # 256 v feats per core
HID = EMB * BD     # 4096
P = 128
K = 16             # chunks
NHO = P // K       # 8 head-groups on partitions
NHR = HL // NHO    # 4 heads per group (free dim)
JW = BD + 1        # 9 jcols of ST ([T|u]^T)
STW = NHR * JW * BD  # 288 = per-partition ST row
NT = SEQ // P      # 16 token tiles
QT = 512
NQ = SEQ // QT     # 4
TPQ = QT // P      # 4
NSL = 4            # 512-wide feat slices
SL = NF // NSL     # 512
HSL = HL // NSL    # 8 heads per slice
CHK = SEQ // K     # 128 positions per chunk

N_CORES = 8
ACT_SET = 6        # natural_log_exp_and_others: ln/exp/relu/square/copy

# tuning knobs
MULT_SPLIT_HR = False   # True: 4 smaller 4-dim mults instead of one 5-dim
PC_POOL_MOD = 4         # phase-C tiles with tau % PC_POOL_MOD == 3 go to Pool


def build_nc():
    nc = bacc.Bacc("TRN2", target_bir_lowering=False, debug=False)

    xT = nc.dram_tensor("xT", [EMB, SEQ], BF16, kind="ExternalInput")
    w1 = nc.dram_tensor("w1", [EMB, HID], BF16, kind="ExternalInput")
    b1 = nc.dram_tensor("b1", [HID, 1], F32, kind="ExternalInput")
    w2 = nc.dram_tensor("w2", [HID, NF], BF16, kind="ExternalInput")
    b2 = nc.dram_tensor("b2", [1, NF], BF16, kind="ExternalInput")
    v1 = nc.dram_tensor("v1", [EMB, EMB], BF16, kind="ExternalInput")
    c1 = nc.dram_tensor("c1", [EMB, 1], F32, kind="ExternalInput")
    v2 = nc.dram_tensor("v2", [EMB, VF], BF16, kind="ExternalInput")
    c2 = nc.dram_tensor("c2", [1, VF], BF16, kind="ExternalInput")
    a0 = nc.dram_tensor("a0", [NHO, NHR * BD], F16, kind="ExternalInput")
    out = nc.dram_tensor("out", [SEQ, VF], F32, kind="ExternalOutput")

    # scratch in scan order: [tau][c][ho][jr][payload]
    a_dram = nc.dram_tensor("a_scratch", [NT * P * NF], F16)
    v_dram = nc.dram_tensor("v_scratch", [NT * P * VF], F16)
    tu_dram = nc.dram_tensor("tu_scratch", [NT * P * BD * STW], F16)

    with ExitStack() as ctx:
        tc = ctx.enter_context(tile.TileContext(nc))
        cpool = ctx.enter_context(tc.tile_pool(name="consts", bufs=1))
        wpool = ctx.enter_context(tc.tile_pool(name="weights", bufs=1))
        xpool = ctx.enter_context(tc.tile_pool(name="xstream", bufs=2))
        hpool = ctx.enter_context(tc.tile_pool(name="hidden", bufs=1))
        hvpool = ctx.enter_context(tc.tile_pool(name="hv", bufs=1))
        w2pool = ctx.enter_context(tc.tile_pool(name="w2stream", bufs=3))
        l1ps = ctx.enter_context(tc.tile_pool(name="l1ps", bufs=2, space="PSUM"))
        l2ps = ctx.enter_context(tc.tile_pool(name="l2ps", bufs=TPQ, space="PSUM"))
        vps = ctx.enter_context(tc.tile_pool(name="vps", bufs=1, space="PSUM"))
        blkpool = ctx.enter_context(tc.tile_pool(name="blk", bufs=4))
        pwpool = ctx.enter_context(tc.tile_pool(name="pw", bufs=2))
        smpool = ctx.enter_context(tc.tile_pool(name="small", bufs=4))
        aipool = ctx.enter_context(tc.tile_pool(name="ai", bufs=3))
        vtpool = ctx.enter_context(tc.tile_pool(name="vtile", bufs=2))
        agpool = ctx.enter_context(tc.tile_pool(name="ag", bufs=2))
        vgpool = ctx.enter_context(tc.tile_pool(name="vg", bufs=2))
        tupool = ctx.enter_context(tc.tile_pool(name="tu", bufs=2))
        mopool = ctx.enter_context(tc.tile_pool(name="mo", bufs=2))
        scpool = ctx.enter_context(tc.tile_pool(name="scan", bufs=1))
        tcpool = ctx.enter_context(tc.tile_pool(name="tuc", bufs=3))
        tcpoolB = ctx.enter_context(tc.tile_pool(name="tucB", bufs=1))
        pcpoolB = ctx.enter_context(tc.tile_pool(name="pcB", bufs=1))

        nc.scalar.add_instruction(mybir.InstLoadActFuncSet(
            name=nc.get_next_instruction_name(), act_func_set_id=ACT_SET,
            ins=[], outs=[]))

        # ---- constants / weights ----
        ones_s = cpool.tile([1, P], BF16, tag="ones")
        nc.vector.memset(ones_s[:], 1.0)
        b1_s = cpool.tile([P, HID // P], F32, tag="b1")
        nc.sync.dma_start(b1_s[:], b1[:].rearrange("(m p) one -> p (m one)", p=P))
        c1_s = cpool.tile([P, EMB // P], F32, tag="c1")
        nc.sync.dma_start(c1_s[:], c1[:].rearrange("(m p) one -> p (m one)", p=P))
        b2_s = cpool.tile([1, NF], BF16, tag="b2")
        nc.sync.dma_start(b2_s[:], b2[:])
        c2_s = cpool.tile([1, VF], BF16, tag="c2")
        nc.sync.dma_start(c2_s[:], c2[:])
        a0_s = cpool.tile([NHO, NHR * BD], F16, tag="a0")
        nc.sync.dma_start(a0_s[:], a0[:])

        w1_s = wpool.tile([P, 4, HID], BF16, tag="w1")
        for k in range(4):
            nc.sync.dma_start(
                w1_s[:, k, bass.ds(0, HID // 4)],
                bass.AP(w1, k * P * HID,
                        [[HID, P], [1, HID // 4]]))
        xq0 = xpool.tile([P, 4, QT], BF16, tag="xq", name="xq_pre0")
        for ti in range(4):
            for k in range(4):
                nc.sync.dma_start(
                    xq0[:, k, bass.ts(ti, P)],
                    bass.AP(xT, k * P * SEQ + ti * 8,
                            [[SEQ, P], [CHK, K], [1, 8]]))
        for k in range(4):
            for mg in range(1, 4):
                nc.sync.dma_start(
                    w1_s[:, k, bass.ds(mg * HID // 4, HID // 4)],
                    bass.AP(w1, k * P * HID + mg * HID // 4,
                            [[HID, P], [1, HID // 4]]))
        v1_s = wpool.tile([P, 4, EMB], BF16, tag="v1")
        nc.sync.dma_start(v1_s[:], v1[:].rearrange("(k p) m -> p k m", p=P))
        v2_s = wpool.tile([P, 4, VF], BF16, tag="v2")
        nc.sync.dma_start(v2_s[:], v2[:].rearrange("(k p) n -> p k n", p=P))

        # ================= phase-1 =================
        # per partition (c,ho), per hr: ST[jcol, i] fp16 (jcol 9, i 8)
        # step: ST_new[jcol,i] = sum_k ST[jcol,k]*A[i,k]; u row += v
        tu_box = {}

        def p1_step(tau, jr, a8, v8, tu8):
            if tau == 0 and jr == 0:
                nc.vector.tensor_copy(
                    bass.AP(tu8.tensor, tu8[:].offset,
                            [[BD * STW, P], [JW * BD, NHR], [1, BD], [BD, BD]]),
                    bass.AP(a8.tensor, a8[:].offset,
                            [[BD * NHR * BD * BD, P], [BD * BD, NHR],
                             [BD, BD], [1, BD]]))
                nc.vector.tensor_copy(
                    bass.AP(tu8.tensor, tu8[:].offset + BD * BD,
                            [[BD * STW, P], [JW * BD, NHR], [1, BD]]),
                    bass.AP(v8.tensor, v8[:].offset,
                            [[BD * NHR * BD, P], [BD, NHR], [1, BD]]))
                return
            if jr == 0:
                src_t, src_off = tu_box['prev'], (BD - 1) * STW
            else:
                src_t, src_off = tu8, (jr - 1) * STW
            mo = mopool.tile([P, NHR * JW * BD * BD], F16, tag="mo",
                             name=f"mo{tau}_{jr}")
            # mo[hr, jcol, i, k] = ST[hr, jcol, k] * A[hr, i, k]
            if MULT_SPLIT_HR:
                for hr in range(NHR):
                    nc.vector.tensor_tensor(
                        bass.AP(mo.tensor, mo[:].offset + hr * JW * BD * BD,
                                [[NHR * JW * BD * BD, P], [BD * BD, JW],
                                 [BD, BD], [1, BD]]),
                        bass.AP(src_t.tensor,
                                src_t[:].offset + src_off + hr * JW * BD,
                                [[BD * STW, P], [BD, JW], [0, BD], [1, BD]]),
                        bass.AP(a8.tensor,
                                a8[:].offset + jr * NHR * BD * BD + hr * BD * BD,
                                [[BD * NHR * BD * BD, P], [0, JW],
                                 [BD, BD], [1, BD]]),
                        ALU.mult)
            else:
                nc.vector.tensor_tensor(
                    bass.AP(mo.tensor, mo[:].offset,
                            [[NHR * JW * BD * BD, P], [JW * BD * BD, NHR],
                             [BD * BD, JW], [BD, BD], [1, BD]]),
                    bass.AP(src_t.tensor, src_t[:].offset + src_off,
                            [[BD * STW, P], [JW * BD, NHR],
                             [BD, JW], [0, BD], [1, BD]]),
                    bass.AP(a8.tensor, a8[:].offset + jr * NHR * BD * BD,
                            [[BD * NHR * BD * BD, P], [BD * BD, NHR],
                             [0, JW], [BD, BD], [1, BD]]),
                    ALU.mult)
            m1 = mopool.tile([P, NHR * JW * BD * 4], F16, tag="m1",
                             name=f"m1{tau}_{jr}")
            nc.vector.tensor_tensor(
                m1[:],
                bass.AP(mo.tensor, mo[:].offset,
                        [[NHR * JW * BD * BD, P], [BD, NHR * JW * BD], [1, 4]]),
                bass.AP(mo.tensor, mo[:].offset + 4,
                        [[NHR * JW * BD * BD, P], [BD, NHR * JW * BD], [1, 4]]),
                ALU.add)
            m2 = mopool.tile([P, NHR * JW * BD * 2], F16, tag="m2",
                             name=f"m2{tau}_{jr}")
            nc.vector.tensor_tensor(
                m2[:],
                bass.AP(m1.tensor, m1[:].offset,
                        [[NHR * JW * BD * 4, P], [4, NHR * JW * BD], [1, 2]]),
                bass.AP(m1.tensor, m1[:].offset + 2,
                        [[NHR * JW * BD * 4, P], [4, NHR * JW * BD], [1, 2]]),
                ALU.add)
            nc.vector.tensor_tensor(
                bass.AP(tu8.tensor, tu8[:].offset + jr * STW,
                        [[BD * STW, P], [1, STW]]),
                bass.AP(m2.tensor, m2[:].offset,
                        [[NHR * JW * BD * 2, P], [2, STW]]),
                bass.AP(m2.tensor, m2[:].offset + 1,
                        [[NHR * JW * BD * 2, P], [2, STW]]),
                ALU.add)
            urow = bass.AP(tu8.tensor, tu8[:].offset + jr * STW + BD * BD,
                           [[BD * STW, P], [JW * BD, NHR], [1, BD]])
            nc.vector.tensor_tensor(
                urow, urow,
                bass.AP(v8.tensor, v8[:].offset + jr * NHR * BD,
                        [[BD * NHR * BD, P], [BD, NHR], [1, BD]]),
                ALU.add)

        def p1_tile(tau):
            if tau >= NT - 2:
                tu_box[f'tu8_{tau}'] = None  # filled below
            a8 = agpool.tile([P, BD, NHR * BD * BD], F16, tag="a8",
                             name=f"a8_{tau}")
            v8 = vgpool.tile([P, BD, NHR * BD], F16, tag="v8", name=f"v8_{tau}")
            geng = nc.gpsimd
            geng.dma_start(a8[:], bass.AP(
                a_dram, tau * P * NF, [[BD * 256, P], [1, BD * 256]]))
            geng.dma_start(v8[:], bass.AP(
                v_dram, tau * P * VF, [[BD * 32, P], [1, BD * 32]]))
            tu8 = tupool.tile([P, BD, STW], F16, tag="tu8", name=f"tu8_{tau}")
            for jr in range(BD):
                p1_step(tau, jr, a8, v8, tu8)
            tu_box['prev'] = tu8
            tu_box[f'tu8_{tau}'] = tu8
            nc.scalar.dma_start(
                bass.AP(tu_dram, tau * P * BD * STW,
                        [[BD * STW, P], [1, BD * STW]]),
                tu8[:])

        # ================= stage A =================
        SCHED = [4, 4, 4, 2, 2]  # tiles per q-group (tapered tail)
        t0 = 0
        for qi, nt_q in enumerate(SCHED):
            W = nt_q * P
            if qi == 0:
                xq = xq0
            else:
                xq = xpool.tile([P, 4, QT], BF16, tag="xq", name=f"xq{qi}")
                for ti in range(nt_q):
                    for k in range(4):
                        nc.sync.dma_start(
                            xq[:, k, bass.ts(ti, P)],
                            bass.AP(xT, k * P * SEQ + (t0 + ti) * 8,
                                    [[SEQ, P], [CHK, K], [1, 8]]))

            hid_t = hpool.tile([P, HID // P, QT], BF16, tag="hid",
                               name=f"hid{qi}")
            for m in range(HID // P):
                ps = l1ps.tile([P, QT], F32, tag="l1")
                for k in range(4):
                    nc.tensor.matmul(ps[:, :W], w1_s[:, k, bass.ts(m, P)],
                                     xq[:, k, :W], start=(k == 0),
                                     stop=(k == 3))
                nc.scalar.activation(hid_t[:, m, :W], ps[:, :W], AF.Relu,
                                     bias=b1_s[:, m:m + 1])

            hv_t = hvpool.tile([P, 4, QT], BF16, tag="hv", name=f"hv{qi}")
            for m in range(4):
                ps = l1ps.tile([P, QT], F32, tag="l1")
                for k in range(4):
                    nc.tensor.matmul(ps[:, :W], v1_s[:, k, bass.ts(m, P)],
                                     xq[:, k, :W], start=(k == 0),
                                     stop=(k == 3))
                nc.scalar.activation(hv_t[:, m, :W], ps[:, :W], AF.Relu,
                                     bias=c1_s[:, m:m + 1])

            # ---- L2 + per-slice norm ----
            for n in range(NSL):
                pss = [l2ps.tile([P, SL], F32, tag="l2",
                                 name=f"l2_{qi}_{n}_{i}")
                       for i in range(nt_q)]
                for ti in range(nt_q):
                    nc.tensor.matmul(pss[ti][:], ones_s[:1, :],
                                     b2_s[:1, bass.ts(n, SL)],
                                     start=True, stop=False)
                for kg in range(8):
                    w2s = w2pool.tile([P, 4, SL], BF16, tag="w2s",
                                      name=f"w2_{qi}_{n}_{kg}")
                    nc.sync.dma_start(w2s[:], bass.AP(
                        w2, kg * 4 * P * NF + n * SL,
                        [[NF, P], [P * NF, 4], [1, SL]]))
                    for k8 in range(4):
                        k = kg * 4 + k8
                        for ti in range(nt_q):
                            nc.tensor.matmul(
                                pss[ti][:], hid_t[:, k, bass.ts(ti, P)],
                                w2s[:, k8, :], start=False,
                                stop=(k == HID // P - 1))
                # psum-freeing copies first, then the norm chains
                blks = []
                for ti in range(nt_q):
                    blk = blkpool.tile([P, SL], F32, tag="blk",
                                       name=f"blk{qi}_{n}_{ti}")
                    nc.scalar.activation(blk[:], pss[ti][:], AF.Identity)
                    blks.append(blk)
                for ti in range(nt_q):
                    blk = blks[ti]
                    pw = pwpool.tile([P, SL], F32, tag="pw")
                    nc.scalar.activation(pw[:], blk[:], AF.Square)
                    nc.scalar.activation(pw[:], pw[:], AF.Ln)
                    nc.scalar.activation(pw[:], pw[:], AF.Exp, scale=0.6)
                    pst = smpool.tile([P, HSL * BD], F32, tag="pst")
                    nc.vector.tensor_reduce(
                        pst[:].rearrange("p (h j) -> p h j", h=HSL, j=BD),
                        bass.AP(pw.tensor, pw[:].offset,
                                [[SL, P], [64, HSL], [1, BD], [8, BD]]),
                        axis=mybir.AxisListType.X, op=ALU.add)
                    dm = smpool.tile([P, HSL], F32, tag="dm")
                    nc.vector.tensor_reduce(
                        dm[:].rearrange("p (h one) -> p h one", h=HSL, one=1),
                        pst[:].rearrange("p (h j) -> p h j", h=HSL, j=BD),
                        axis=mybir.AxisListType.X, op=ALU.max)
                    rc = smpool.tile([P, HSL], F32, tag="rc")
                    nc.scalar.activation(rc[:], dm[:], AF.Ln)
                    nc.scalar.activation(rc[:], rc[:], AF.Exp,
                                         scale=-1.0 / 1.2)
                    asl = aipool.tile([P, SL], F16, tag="ai",
                                      name=f"ai{qi}_{n}_{ti}")
                    nc.gpsimd.tensor_tensor(
                        asl[:].rearrange("p (h i k) -> p h i k",
                                         h=HSL, i=BD, k=BD),
                        blk[:].rearrange("p (h i k) -> p h i k",
                                         h=HSL, i=BD, k=BD),
                        bass.AP(rc.tensor, rc[:].offset,
                                [[HSL, P], [1, HSL], [0, BD], [0, BD]]),
                        ALU.mult)
                    tt = t0 + ti
                    weng = nc.gpsimd
                    for ho2 in range(2):
                        # dst (tau, c, ho=2n+ho2, jr=j, (hr,i,k))
                        weng.dma_start(
                            bass.AP(a_dram,
                                    tt * P * NF + (2 * n + ho2) * BD * 256,
                                    [[NHO * BD * 256, K], [256, BD],
                                     [1, 256]]),
                            bass.AP(asl.tensor, asl[:].offset + ho2 * 256,
                                    [[SL, P], [1, 256]]))

            for ti in range(nt_q):
                tt = t0 + ti
                psv = vps.tile([P, VF], F32, tag="v")
                nc.tensor.matmul(psv[:], ones_s[:1, :], c2_s[:1, :],
                                 start=True, stop=False)
                for k in range(4):
                    nc.tensor.matmul(psv[:], hv_t[:, k, bass.ts(ti, P)],
                                     v2_s[:, k, :], start=False,
                                     stop=(k == 3))
                vt = vtpool.tile([P, VF], F16, tag="vt")
                nc.scalar.activation(vt[:], psv[:], AF.Identity)
                veng = nc.gpsimd
                for ho in range(NHO):
                    veng.dma_start(
                        bass.AP(v_dram, tt * P * VF + ho * BD * 32,
                                [[NHO * BD * 32, K], [32, BD], [1, 32]]),
                        bass.AP(vt.tensor, vt[:].offset + ho * 32,
                                [[VF, P], [1, 32]]))

                p1_tile(tt)
            t0 += nt_q

        # ================= phase B =================
        # extract per-chunk summaries from tu8 of the last tile (SBUF->SBUF)
        tu_last = tu_box['prev']
        tub = scpool.tile([NHO, K * STW], F16, tag="tub")
        for c in range(K):
            qeng = nc.sync if c % 2 == 0 else nc.scalar
            qeng.dma_start(
                bass.AP(tub.tensor, tub[:].offset + c * STW,
                        [[K * STW, NHO], [1, STW]]),
                bass.AP(tu_last.tensor,
                        tu_last[:].offset + c * NHO * BD * STW
                        + (BD - 1) * STW,
                        [[BD * STW, NHO], [1, STW]]))
        sg = scpool.tile([NHO, (K + 1) * NHR * BD], F16, tag="sg")
        nc.vector.tensor_copy(sg[:, 0:NHR * BD], a0_s[:])
        for c in range(K):
            mB = smpool.tile([NHO, NHR * BD * BD], F32, tag="mB", name=f"mB{c}")
            nc.vector.tensor_tensor(
                bass.AP(mB.tensor, mB[:].offset,
                        [[NHR * BD * BD, NHO], [BD * BD, NHR],
                         [BD, BD], [1, BD]]),
                bass.AP(tub.tensor, tub[:].offset + c * STW,
                        [[K * STW, NHO], [JW * BD, NHR], [BD, BD], [1, BD]]),
                bass.AP(sg.tensor, sg[:].offset + c * NHR * BD,
                        [[(K + 1) * NHR * BD, NHO], [BD, NHR],
                         [1, BD], [0, BD]]),
                ALU.mult)
            sn = smpool.tile([NHO, NHR * BD], F32, tag="sn", name=f"sn{c}")
            nc.vector.tensor_reduce(
                bass.AP(sn.tensor, sn[:].offset,
                        [[NHR * BD, NHO], [BD, NHR], [1, BD]]),
                bass.AP(mB.tensor, mB[:].offset,
                        [[NHR * BD * BD, NHO], [BD * BD, NHR],
                         [1, BD], [BD, BD]]),
                axis=mybir.AxisListType.X, op=ALU.add)
            nc.vector.tensor_tensor(
                bass.AP(sg.tensor, sg[:].offset + (c + 1) * NHR * BD,
                        [[(K + 1) * NHR * BD, NHO], [1, NHR * BD]]),
                bass.AP(sn.tensor, sn[:].offset,
                        [[NHR * BD, NHO], [1, NHR * BD]]),
                bass.AP(tub.tensor, tub[:].offset + c * STW + BD * BD,
                        [[K * STW, NHO], [JW * BD, NHR], [1, BD]]),
                ALU.add)

        # scatter sigma_c -> sc[(c,ho), (hr,k)], then expand over i
        sc_t = scpool.tile([P, NHR * BD], F16, tag="sc")
        for ho in range(NHO):
            nc.sync.dma_start(
                sc_t[ho::NHO, :],
                bass.AP(sg.tensor, sg[:].offset + ho * (K + 1) * NHR * BD,
                        [[(K + 1) * NHR * BD, 1], [NHR * BD, K],
                         [1, NHR * BD]]))
        sexp = scpool.tile([P, NHR * BD * BD], F16, tag="sexp")
        nc.vector.tensor_copy(
            bass.AP(sexp.tensor, sexp[:].offset,
                    [[NHR * BD * BD, P], [BD, NHR * BD], [1, BD]]),
            bass.AP(sc_t.tensor, sc_t[:].offset,
                    [[NHR * BD, P], [1, NHR * BD], [0, BD]]))

        # ================= phase C =================
        NTD = NT - 4  # DVE handles tau 0..11, Pool 12..15
        s_outA = scpool.tile([P, NTD * BD * NHR * BD], F32, tag="soutA")
        s_outB = scpool.tile([P, 4 * BD * NHR * BD], F32, tag="soutB")
        PCW = BD * NHR * BD * BD  # 2048
        for tau in range(NT):
            on_pool = (tau >= NTD)
            eng = nc.gpsimd if on_pool else nc.vector
            if tau >= NT - 2 and f'tu8_{tau}' in tu_box:
                tuc = tu_box[f'tu8_{tau}']  # still live in SBUF
            else:
                tpool = tcpoolB if on_pool else tcpool
                tuc = tpool.tile([P, BD, STW], F16, tag="tuc",
                                 name=f"tuc{tau}")
                nc.sync.dma_start(tuc[:], bass.AP(
                    tu_dram, tau * P * BD * STW,
                    [[BD * STW, P], [1, BD * STW]]))
            if on_pool:
                m = pcpoolB.tile([P, PCW], F16, tag="pcm", name=f"pcm{tau}")
                m1 = pcpoolB.tile([P, PCW // 2], F16, tag="pcm1",
                                  name=f"pcm1{tau}")
                m2 = pcpoolB.tile([P, PCW // 4], F16, tag="pcm2",
                                  name=f"pcm2{tau}")
                m3 = pcpoolB.tile([P, PCW // 8], F16, tag="pcm3",
                                  name=f"pcm3{tau}")
            else:
                m = mopool.tile([P, NHR * JW * BD * BD], F16, tag="mo",
                                name=f"pcm{tau}")
                m1 = mopool.tile([P, NHR * JW * BD * 4], F16, tag="m1",
                                 name=f"pcm1{tau}")
                m2 = mopool.tile([P, NHR * JW * BD * 2], F16, tag="m2",
                                 name=f"pcm2{tau}")
                m3 = mopool.tile([P, NHR * JW * BD * BD], F16, tag="mo",
                                 name=f"pcm3{tau}")
            MP = m[:].ap[0][0]
            M1P = m1[:].ap[0][0]
            M2P = m2[:].ap[0][0]
            M3P = m3[:].ap[0][0]
            # m[jr, hr, (k,i)] = ST[jr,hr,jcol=k,i] * sigma_exp[hr,(k,i)]
            eng.tensor_tensor(
                bass.AP(m.tensor, m[:].offset,
                        [[MP, P], [NHR * BD * BD, BD],
                         [BD * BD, NHR], [1, BD * BD]]),
                bass.AP(tuc.tensor, tuc[:].offset,
                        [[BD * STW, P], [STW, BD], [JW * BD, NHR],
                         [1, BD * BD]]),
                bass.AP(sexp.tensor, sexp[:].offset,
                        [[NHR * BD * BD, P], [0, BD], [BD * BD, NHR],
                         [1, BD * BD]]),
                ALU.mult)
            eng.tensor_tensor(
                bass.AP(m1.tensor, m1[:].offset, [[M1P, P], [1, PCW // 2]]),
                bass.AP(m.tensor, m[:].offset,
                        [[MP, P], [BD * BD, BD * NHR], [1, 4 * BD]]),
                bass.AP(m.tensor, m[:].offset + 4 * BD,
                        [[MP, P], [BD * BD, BD * NHR], [1, 4 * BD]]),
                ALU.add)
            eng.tensor_tensor(
                bass.AP(m2.tensor, m2[:].offset, [[M2P, P], [1, PCW // 4]]),
                bass.AP(m1.tensor, m1[:].offset,
                        [[M1P, P], [4 * BD, BD * NHR], [1, 2 * BD]]),
                bass.AP(m1.tensor, m1[:].offset + 2 * BD,
                        [[M1P, P], [4 * BD, BD * NHR], [1, 2 * BD]]),
                ALU.add)
            eng.tensor_tensor(
                bass.AP(m3.tensor, m3[:].offset, [[M3P, P], [1, PCW // 8]]),
                bass.AP(m2.tensor, m2[:].offset,
                        [[M2P, P], [2 * BD, BD * NHR], [1, BD]]),
                bass.AP(m2.tensor, m2[:].offset + BD,
                        [[M2P, P], [2 * BD, BD * NHR], [1, BD]]),
                ALU.add)
            # s = m3 + u  -> s_out half (f32)
            s_o = s_outB if on_pool else s_outA
            s_off = (tau - NTD if on_pool else tau) * BD * NHR * BD
            eng.tensor_tensor(
                bass.AP(s_o.tensor, s_o[:].offset + s_off,
                        [[s_o[:].ap[0][0], P], [1, BD * NHR * BD]]),
                bass.AP(m3.tensor, m3[:].offset, [[M3P, P], [1, PCW // 8]]),
                bass.AP(tuc.tensor, tuc[:].offset + BD * BD,
                        [[BD * STW, P], [STW, BD], [JW * BD, NHR], [1, BD]]),
                ALU.add)

        # ---- output: s_out halves [(c,ho), (pos, hr, i)] -> out[t, vf] ----
        for c in range(K):
            nc.sync.dma_start(
                bass.AP(out, c * CHK * VF,
                        [[NHR * BD, NHO], [VF, NTD * BD], [1, NHR * BD]]),
                bass.AP(s_outA.tensor,
                        s_outA[c * NHO:(c + 1) * NHO, :].offset,
                        [[NTD * BD * NHR * BD, NHO], [NHR * BD, NTD * BD],
                         [1, NHR * BD]]))
            nc.scalar.dma_start(
                bass.AP(out, (c * CHK + NTD * BD) * VF,
                        [[NHR * BD, NHO], [VF, 4 * BD], [1, NHR * BD]]),
                bass.AP(s_outB.tensor,
                        s_outB[c * NHO:(c + 1) * NHO, :].offset,
                        [[4 * BD * NHR * BD, NHO], [NHR * BD, 4 * BD],
                         [1, NHR * BD]]))

    nc.compile()
    return nc


# ---------------- host side ----------------

_NC_CACHE = {}


def _get_nc(TOK=SEQ):
    if TOK not in _NC_CACHE:
        _NC_CACHE[TOK] = build_nc()
    return _NC_CACHE[TOK]


def prep_shared(W1, b1, W2, b2, V1, c1, V2, c2, a0):
    bf = ml_dtypes.bfloat16
    W2r = np.asarray(W2).reshape(H, BD, BD, HID)
    W2c = (W2r - W2r.mean(axis=1, keepdims=True)).reshape(H * BD * BD, HID)
    b2r = np.asarray(b2).reshape(H, BD, BD)
    b2c = (b2r - b2r.mean(axis=1, keepdims=True)).reshape(-1)
    shared = {
        "w1": np.ascontiguousarray(np.asarray(W1).T).astype(bf),
        "b1": np.asarray(b1).reshape(HID, 1).astype(np.float32),
        "v1": np.ascontiguousarray(np.asarray(V1).T).astype(bf),
        "c1": np.asarray(c1).reshape(EMB, 1).astype(np.float32),
    }
    halves = []
    for half in range(2):
        rsl = slice(half * NF, (half + 1) * NF)
        vsl = slice(half * VF, (half + 1) * VF)
        hsl = slice(half * HL, (half + 1) * HL)
        a0h = np.asarray(a0)[0, hsl]                       # [32, 8]
        a0p = a0h.reshape(NHO, NHR * BD)                   # [8, 32]
        halves.append({
            "w2": np.ascontiguousarray(W2c[rsl].T).astype(bf),
            "b2": b2c[rsl].reshape(1, NF).astype(bf),
            "v2": np.ascontiguousarray(np.asarray(V2)[vsl].T).astype(bf),
            "c2": np.asarray(c2)[vsl].reshape(1, VF).astype(bf),
            "a0": a0p.astype(np.float16),
        })
    return shared, halves


def make_in_maps(x, W1, b1, W2, b2, V1, c1, V2, c2, a0):
    shared, halves = prep_shared(W1, b1, W2, b2, V1, c1, V2, c2, a0)
    bf = ml_dtypes.bfloat16
    in_maps = []
    for core in range(N_CORES):
        b, half = core // 2, core % 2
        m = dict(shared)
        m.update(halves[half])
        m["xT"] = np.ascontiguousarray(np.asarray(x)[b].T).astype(bf)
        in_maps.append(m)
    return in_maps


def kernel(x, W1, b1, W2, b2, V1, c1, V2, c2, a0):
    from concourse import bass_utils
    nc = _get_nc(SEQ)
    in_maps = make_in_maps(x, W1, b1, W2, b2, V1, c1, V2, c2, a0)
    res = bass_utils.run_bass_kernel_spmd(nc, in_maps,
                                          core_ids=list(range(N_CORES)))
    out = np.zeros((BS, SEQ, EMB), np.float32)
    for core in range(N_CORES):
        b, half = core // 2, core % 2
        out[b, :, half * VF:(half + 1) * VF] = res.results[core]["out"]
    return out


# revision 4
# speedup vs baseline: 1.5810x; 1.0051x over previous
"""Trainium2 Bass kernel v2 for nn_BlockModel_82678120448388.

Per core (batch b, head-half): MLPs on PE (bf16, W1 resident, batched W2
stream), p=1.2 norm per 512-feat slice (Act + Pool reduce, one act table),
chunked scan (K=16 chunks x 128 steps) on DVE in fp16 with 2x_1p layouts:
state ST[jcol,i] = [T|u]^T updated per step via mult + k-tree adds.
Per-step [T|u] stored to DRAM (fp16); phase C is bulk-parallel
s_t = u_t + T_t sigma_c (split DVE/Pool, no serial tail).
"""

import numpy as np
import ml_dtypes
from contextlib import ExitStack

import concourse.bass as bass
import concourse.bacc as bacc
import concourse.tile as tile
from concourse import mybir

F32 = mybir.dt.float32
F16 = mybir.dt.float16
BF16 = mybir.dt.bfloat16
AF = mybir.ActivationFunctionType
ALU = mybir.AluOpType

BS, SEQ, EMB, BD = 4, 2048, 512, 8
H = EMB // BD
HL = 32            # heads per core
NF = HL * BD * BD  # 2048 blk feats per core
VF = HL * BD       # 256 v feats per core
HID = EMB * BD     # 4096
P = 128
K = 16             # chunks
NHO = P // K       # 8 head-groups on partitions
NHR = HL // NHO    # 4 heads per group (free dim)
JW = BD + 1        # 9 jcols of ST ([T|u]^T)
STW = NHR * JW * BD  # 288 = per-partition ST row
NT = SEQ // P      # 16 token tiles
QT = 512
NQ = SEQ // QT     # 4
TPQ = QT // P      # 4
NSL = 4            # 512-wide feat slices
SL = NF // NSL     # 512
HSL = HL // NSL    # 8 heads per slice
CHK = SEQ // K     # 128 positions per chunk

N_CORES = 8
ACT_SET = 6        # natural_log_exp_and_others: ln/exp/relu/square/copy

# tuning knobs
MULT_SPLIT_HR = False   # True: 4 smaller 4-dim mults instead of one 5-dim
PC_POOL_MOD = 4         # phase-C tiles with tau % PC_POOL_MOD == 3 go to Pool


def build_nc():
    nc = bacc.Bacc("TRN2", target_bir_lowering=False, debug=False)

    xT = nc.dram_tensor("xT", [EMB, SEQ], BF16, kind="ExternalInput")
    w1 = nc.dram_tensor("w1", [EMB, HID], BF16, kind="ExternalInput")
    b1 = nc.dram_tensor("b1", [HID, 1], F32, kind="ExternalInput")
    w2 = nc.dram_tensor("w2", [HID, NF], BF16, kind="ExternalInput")
    b2 = nc.dram_tensor("b2", [1, NF], BF16, kind="ExternalInput")
    v1 = nc.dram_tensor("v1", [EMB, EMB], BF16, kind="ExternalInput")
    c1 = nc.dram_tensor("c1", [EMB, 1], F32, kind="ExternalInput")
    v2 = nc.dram_tensor("v2", [EMB, VF], BF16, kind="ExternalInput")
    c2 = nc.dram_tensor("c2", [1, VF], BF16, kind="ExternalInput")
    a0 = nc.dram_tensor("a0", [NHO, NHR * BD], F16, kind="ExternalInput")
    out = nc.dram_tensor("out", [SEQ, VF], F32, kind="ExternalOutput")

    # scratch in scan order: [tau][c][ho][jr][payload]
    a_dram = nc.dram_tensor("a_scratch", [NT * P * NF], F16)
    v_dram = nc.dram_tensor("v_scratch", [NT * P * VF], F16)
    tu_dram = nc.dram_tensor("tu_scratch", [NT * P * BD * STW], F16)

    with ExitStack() as ctx:
        tc = ctx.enter_context(tile.TileContext(nc))
        cpool = ctx.enter_context(tc.tile_pool(name="consts", bufs=1))
        wpool = ctx.enter_context(tc.tile_pool(name="weights", bufs=1))
        xpool = ctx.enter_context(tc.tile_pool(name="xstream", bufs=2))
        hpool = ctx.enter_context(tc.tile_pool(name="hidden", bufs=1))
        hvpool = ctx.enter_context(tc.tile_pool(name="hv", bufs=1))
        w2pool = ctx.enter_context(tc.tile_pool(name="w2stream", bufs=3))
        l1ps = ctx.enter_context(tc.tile_pool(name="l1ps", bufs=2, space="PSUM"))
        l2ps = ctx.enter_context(tc.tile_pool(name="l2ps", bufs=TPQ, space="PSUM"))
        vps = ctx.enter_context(tc.tile_pool(name="vps", bufs=2, space="PSUM"))
        blkpool = ctx.enter_context(tc.tile_pool(name="blk", bufs=4))
        pwpool = ctx.enter_context(tc.tile_pool(name="pw", bufs=2))
        smpool = ctx.enter_context(tc.tile_pool(name="small", bufs=4))
        aipool = ctx.enter_context(tc.tile_pool(name="ai", bufs=3))
        vtpool = ctx.enter_context(tc.tile_pool(name="vtile", bufs=2))
        agpool = ctx.enter_context(tc.tile_pool(name="ag", bufs=2))
        vgpool = ctx.enter_context(tc.tile_pool(name="vg", bufs=2))
        tupool = ctx.enter_context(tc.tile_pool(name="tu", bufs=2))
        mopool = ctx.enter_context(tc.tile_pool(name="mo", bufs=2))
        scpool = ctx.enter_context(tc.tile_pool(name="scan", bufs=1))
        tcpool = ctx.enter_context(tc.tile_pool(name="tuc", bufs=3))
        tcpoolB = ctx.enter_context(tc.tile_pool(name="tucB", bufs=1))
        pcpoolB = ctx.enter_context(tc.tile_pool(name="pcB", bufs=1))

        nc.scalar.add_instruction(mybir.InstLoadActFuncSet(
            name=nc.get_next_instruction_name(), act_func_set_id=ACT_SET,
            ins=[], outs=[]))

        # ---- constants / weights ----
        ones_s = cpool.tile([1, P], BF16, tag="ones")
        nc.vector.memset(ones_s[:], 1.0)
        b1_s = cpool.tile([P, HID // P], F32, tag="b1")
        nc.sync.dma_start(b1_s[:], b1[:].rearrange("(m p) one -> p (m one)", p=P))
        c1_s = cpool.tile([P, EMB // P], F32, tag="c1")
        nc.sync.dma_start(c1_s[:], c1[:].rearrange("(m p) one -> p (m one)", p=P))
        w1_s = wpool.tile([P, 4, HID], BF16, tag="w1")
        for k in range(4):
            nc.sync.dma_start(
                w1_s[:, k, bass.ds(0, HID // 4)],
                bass.AP(w1, k * P * HID,
                        [[HID, P], [1, HID // 4]]))
        xq0 = xpool.tile([P, 4, QT], BF16, tag="xq", name="xq_pre0")
        for k in range(4):
            for ti in range(4):
                nc.sync.dma_start(
                    xq0[:, k, bass.ts(ti, P)],
                    bass.AP(xT, k * P * SEQ + ti * 8,
                            [[SEQ, P], [CHK, K], [1, 8]]))
        b2_s = cpool.tile([1, NF], BF16, tag="b2")
        nc.sync.dma_start(b2_s[:], b2[:])
        c2_s = cpool.tile([1, VF], BF16, tag="c2")
        nc.sync.dma_start(c2_s[:], c2[:])
        a0_s = cpool.tile([NHO, NHR * BD], F16, tag="a0")
        nc.sync.dma_start(a0_s[:], a0[:])
        for k in range(4):
            for mg in range(1, 4):
                nc.sync.dma_start(
                    w1_s[:, k, bass.ds(mg * HID // 4, HID // 4)],
                    bass.AP(w1, k * P * HID + mg * HID // 4,
                            [[HID, P], [1, HID // 4]]))
        v1_s = wpool.tile([P, 4, EMB], BF16, tag="v1")
        nc.sync.dma_start(v1_s[:], v1[:].rearrange("(k p) m -> p k m", p=P))
        v2_s = wpool.tile([P, 4, VF], BF16, tag="v2")
        nc.sync.dma_start(v2_s[:], v2[:].rearrange("(k p) n -> p k n", p=P))

        # ================= phase-1 =================
        # per partition (c,ho), per hr: ST[jcol, i] fp16 (jcol 9, i 8)
        # step: ST_new[jcol,i] = sum_k ST[jcol,k]*A[i,k]; u row += v
        tu_box = {}

        def p1_step(tau, jr, a8, v8, tu8):
            if tau == 0 and jr == 0:
                nc.vector.tensor_copy(
                    bass.AP(tu8.tensor, tu8[:].offset,
                            [[BD * STW, P], [JW * BD, NHR], [1, BD], [BD, BD]]),
                    bass.AP(a8.tensor, a8[:].offset,
                            [[BD * NHR * BD * BD, P], [BD * BD, NHR],
                             [BD, BD], [1, BD]]))
                nc.vector.tensor_copy(
                    bass.AP(tu8.tensor, tu8[:].offset + BD * BD,
                            [[BD * STW, P], [JW * BD, NHR], [1, BD]]),
                    bass.AP(v8.tensor, v8[:].offset,
                            [[BD * NHR * BD, P], [BD, NHR], [1, BD]]))
                return
            if jr == 0:
                src_t, src_off = tu_box['prev'], (BD - 1) * STW
            else:
                src_t, src_off = tu8, (jr - 1) * STW
            mo = mopool.tile([P, NHR * JW * BD * BD], F16, tag="mo",
                             name=f"mo{tau}_{jr}")
            # mo[hr, jcol, i, k] = ST[hr, jcol, k] * A[hr, i, k]
            if MULT_SPLIT_HR:
                for hr in range(NHR):
                    nc.vector.tensor_tensor(
                        bass.AP(mo.tensor, mo[:].offset + hr * JW * BD * BD,
                                [[NHR * JW * BD * BD, P], [BD * BD, JW],
                                 [BD, BD], [1, BD]]),
                        bass.AP(src_t.tensor,
                                src_t[:].offset + src_off + hr * JW * BD,
                                [[BD * STW, P], [BD, JW], [0, BD], [1, BD]]),
                        bass.AP(a8.tensor,
                                a8[:].offset + jr * NHR * BD * BD + hr * BD * BD,
                                [[BD * NHR * BD * BD, P], [0, JW],
                                 [BD, BD], [1, BD]]),
                        ALU.mult)
            else:
                nc.vector.tensor_tensor(
                    bass.AP(mo.tensor, mo[:].offset,
                            [[NHR * JW * BD * BD, P], [JW * BD * BD, NHR],
                             [BD * BD, JW], [BD, BD], [1, BD]]),
                    bass.AP(src_t.tensor, src_t[:].offset + src_off,
                            [[BD * STW, P], [JW * BD, NHR],
                             [BD, JW], [0, BD], [1, BD]]),
                    bass.AP(a8.tensor, a8[:].offset + jr * NHR * BD * BD,
                            [[BD * NHR * BD * BD, P], [BD * BD, NHR],
                             [0, JW], [BD, BD], [1, BD]]),
                    ALU.mult)
            m1 = mopool.tile([P, NHR * JW * BD * 4], F16, tag="m1",
                             name=f"m1{tau}_{jr}")
            nc.vector.tensor_tensor(
                m1[:],
                bass.AP(mo.tensor, mo[:].offset,
                        [[NHR * JW * BD * BD, P], [BD, NHR * JW * BD], [1, 4]]),
                bass.AP(mo.tensor, mo[:].offset + 4,
                        [[NHR * JW * BD * BD, P], [BD, NHR * JW * BD], [1, 4]]),
                ALU.add)
            m2 = mopool.tile([P, NHR * JW * BD * 2], F16, tag="m2",
                             name=f"m2{tau}_{jr}")
            nc.vector.tensor_tensor(
                m2[:],
                bass.AP(m1.tensor, m1[:].offset,
                        [[NHR * JW * BD * 4, P], [4, NHR * JW * BD], [1, 2]]),
                bass.AP(m1.tensor, m1[:].offset + 2,
                        [[NHR * JW * BD * 4, P], [4, NHR * JW * BD], [1, 2]]),
                ALU.add)
            nc.vector.tensor_tensor(
                bass.AP(tu8.tensor, tu8[:].offset + jr * STW,
                        [[BD * STW, P], [1, STW]]),
                bass.AP(m2.tensor, m2[:].offset,
                        [[NHR * JW * BD * 2, P], [2, STW]]),
                bass.AP(m2.tensor, m2[:].offset + 1,
                        [[NHR * JW * BD * 2, P], [2, STW]]),
                ALU.add)
            urow = bass.AP(tu8.tensor, tu8[:].offset + jr * STW + BD * BD,
                           [[BD * STW, P], [JW * BD, NHR], [1, BD]])
            nc.vector.tensor_tensor(
                urow, urow,
                bass.AP(v8.tensor, v8[:].offset + jr * NHR * BD,
                        [[BD * NHR * BD, P], [BD, NHR], [1, BD]]),
                ALU.add)

        def p1_tile(tau):
            if tau >= NT - 2:
                tu_box[f'tu8_{tau}'] = None  # filled below
            a8 = agpool.tile([P, BD, NHR * BD * BD], F16, tag="a8",
                             name=f"a8_{tau}")
            v8 = vgpool.tile([P, BD, NHR * BD], F16, tag="v8", name=f"v8_{tau}")
            geng = nc.gpsimd
            geng.dma_start(a8[:], bass.AP(
                a_dram, tau * P * NF, [[BD * 256, P], [1, BD * 256]]))
            geng.dma_start(v8[:], bass.AP(
                v_dram, tau * P * VF, [[BD * 32, P], [1, BD * 32]]))
            tu8 = tupool.tile([P, BD, STW], F16, tag="tu8", name=f"tu8_{tau}")
            for jr in range(BD):
                p1_step(tau, jr, a8, v8, tu8)
            tu_box['prev'] = tu8
            tu_box[f'tu8_{tau}'] = tu8
            nc.scalar.dma_start(
                bass.AP(tu_dram, tau * P * BD * STW,
                        [[BD * STW, P], [1, BD * STW]]),
                tu8[:])

        # ================= stage A =================
        SCHED = [4, 4, 4, 2, 2]  # tiles per q-group (tapered tail)
        t0 = 0
        for qi, nt_q in enumerate(SCHED):
            W = nt_q * P
            if qi == 0:
                xq = xq0
            else:
                xq = xpool.tile([P, 4, QT], BF16, tag="xq", name=f"xq{qi}")
                for ti in range(nt_q):
                    for k in range(4):
                        nc.sync.dma_start(
                            xq[:, k, bass.ts(ti, P)],
                            bass.AP(xT, k * P * SEQ + (t0 + ti) * 8,
                                    [[SEQ, P], [CHK, K], [1, 8]]))

            hid_t = hpool.tile([P, HID // P, QT], BF16, tag="hid",
                               name=f"hid{qi}")
            for m in range(HID // P):
                ps = l1ps.tile([P, QT], F32, tag="l1")
                for k in range(4):
                    nc.tensor.matmul(ps[:, :W], w1_s[:, k, bass.ts(m, P)],
                                     xq[:, k, :W], start=(k == 0),
                                     stop=(k == 3))
                nc.scalar.activation(hid_t[:, m, :W], ps[:, :W], AF.Relu,
                                     bias=b1_s[:, m:m + 1])

            hv_t = hvpool.tile([P, 4, QT], BF16, tag="hv", name=f"hv{qi}")
            for m in range(4):
                ps = l1ps.tile([P, QT], F32, tag="l1")
                for k in range(4):
                    nc.tensor.matmul(ps[:, :W], v1_s[:, k, bass.ts(m, P)],
                                     xq[:, k, :W], start=(k == 0),
                                     stop=(k == 3))
                nc.scalar.activation(hv_t[:, m, :W], ps[:, :W], AF.Relu,
                                     bias=c1_s[:, m:m + 1])

            # ---- L2 + per-slice norm ----
            for n in range(NSL):
                pss = [l2ps.tile([P, SL], F32, tag="l2",
                                 name=f"l2_{qi}_{n}_{i}")
                       for i in range(nt_q)]
                for ti in range(nt_q):
                    nc.tensor.matmul(pss[ti][:], ones_s[:1, :],
                                     b2_s[:1, bass.ts(n, SL)],
                                     start=True, stop=False)
                for kg in range(8):
                    w2s = w2pool.tile([P, 4, SL], BF16, tag="w2s",
                                      name=f"w2_{qi}_{n}_{kg}")
                    nc.sync.dma_start(w2s[:], bass.AP(
                        w2, kg * 4 * P * NF + n * SL,
                        [[NF, P], [P * NF, 4], [1, SL]]))
                    for k8 in range(4):
                        k = kg * 4 + k8
                        for ti in range(nt_q):
                            nc.tensor.matmul(
                                pss[ti][:], hid_t[:, k, bass.ts(ti, P)],
                                w2s[:, k8, :], start=False,
                                stop=(k == HID // P - 1))
                # psum-freeing copies first, then the norm chains
                blks = []
                for ti in range(nt_q):
                    blk = blkpool.tile([P, SL], F32, tag="blk",
                                       name=f"blk{qi}_{n}_{ti}")
                    nc.scalar.activation(blk[:], pss[ti][:], AF.Identity)
                    blks.append(blk)
                for ti in range(nt_q):
                    blk = blks[ti]
                    pw = pwpool.tile([P, SL], F32, tag="pw")
                    nc.scalar.activation(pw[:], blk[:], AF.Square)
                    nc.scalar.activation(pw[:], pw[:], AF.Ln)
                    nc.scalar.activation(pw[:], pw[:], AF.Exp, scale=0.6)
                    pst = smpool.tile([P, HSL * BD], F32, tag="pst")
                    nc.vector.tensor_reduce(
                        pst[:].rearrange("p (h j) -> p h j", h=HSL, j=BD),
                        bass.AP(pw.tensor, pw[:].offset,
                                [[SL, P], [64, HSL], [1, BD], [8, BD]]),
                        axis=mybir.AxisListType.X, op=ALU.add)
                    dm = smpool.tile([P, HSL], F32, tag="dm")
                    nc.vector.tensor_reduce(
                        dm[:].rearrange("p (h one) -> p h one", h=HSL, one=1),
                        pst[:].rearrange("p (h j) -> p h j", h=HSL, j=BD),
                        axis=mybir.AxisListType.X, op=ALU.max)
                    rc = smpool.tile([P, HSL], F32, tag="rc")
                    nc.scalar.activation(rc[:], dm[:], AF.Ln)
                    nc.scalar.activation(rc[:], rc[:], AF.Exp,
                                         scale=-1.0 / 1.2)
                    asl = aipool.tile([P, SL], F16, tag="ai",
                                      name=f"ai{qi}_{n}_{ti}")
                    nc.gpsimd.tensor_tensor(
                        asl[:].rearrange("p (h i k) -> p h i k",
                                         h=HSL, i=BD, k=BD),
                        blk[:].rearrange("p (h i k) -> p h i k",
                                         h=HSL, i=BD, k=BD),
                        bass.AP(rc.tensor, rc[:].offset,
                                [[HSL, P], [1, HSL], [0, BD], [0, BD]]),
                        ALU.mult)
                    tt = t0 + ti
                    weng = nc.gpsimd
                    for ho2 in range(2):
                        # dst (tau, c, ho=2n+ho2, jr=j, (hr,i,k))
                        weng.dma_start(
                            bass.AP(a_dram,
                                    tt * P * NF + (2 * n + ho2) * BD * 256,
                                    [[NHO * BD * 256, K], [256, BD],
                                     [1, 256]]),
                            bass.AP(asl.tensor, asl[:].offset + ho2 * 256,
                                    [[SL, P], [1, 256]]))

            for ti in range(nt_q):
                tt = t0 + ti
                psv = vps.tile([P, VF], F32, tag="v")
                nc.tensor.matmul(psv[:], ones_s[:1, :], c2_s[:1, :],
                                 start=True, stop=False)
                for k in range(4):
                    nc.tensor.matmul(psv[:], hv_t[:, k, bass.ts(ti, P)],
                                     v2_s[:, k, :], start=False,
                                     stop=(k == 3))
                vt = vtpool.tile([P, VF], F16, tag="vt")
                nc.scalar.activation(vt[:], psv[:], AF.Identity)
                veng = nc.gpsimd
                for ho in range(NHO):
                    veng.dma_start(
                        bass.AP(v_dram, tt * P * VF + ho * BD * 32,
                                [[NHO * BD * 32, K], [32, BD], [1, 32]]),
                        bass.AP(vt.tensor, vt[:].offset + ho * 32,
                                [[VF, P], [1, 32]]))

                p1_tile(tt)
            t0 += nt_q

        # ================= phase B =================
        # extract per-chunk summaries from tu8 of the last tile (SBUF->SBUF)
        tu_last = tu_box['prev']
        tub = scpool.tile([NHO, K * STW], F16, tag="tub")
        for c in range(K):
            qeng = nc.sync if c % 2 == 0 else nc.scalar
            qeng.dma_start(
                bass.AP(tub.tensor, tub[:].offset + c * STW,
                        [[K * STW, NHO], [1, STW]]),
                bass.AP(tu_last.tensor,
                        tu_last[:].offset + c * NHO * BD * STW
                        + (BD - 1) * STW,
                        [[BD * STW, NHO], [1, STW]]))
        sg = scpool.tile([NHO, (K + 1) * NHR * BD], F16, tag="sg")
        nc.vector.tensor_copy(sg[:, 0:NHR * BD], a0_s[:])
        for c in range(K):
            mB = smpool.tile([NHO, NHR * BD * BD], F32, tag="mB", name=f"mB{c}")
            nc.vector.tensor_tensor(
                bass.AP(mB.tensor, mB[:].offset,
                        [[NHR * BD * BD, NHO], [BD * BD, NHR],
                         [BD, BD], [1, BD]]),
                bass.AP(tub.tensor, tub[:].offset + c * STW,
                        [[K * STW, NHO], [JW * BD, NHR], [BD, BD], [1, BD]]),
                bass.AP(sg.tensor, sg[:].offset + c * NHR * BD,
                        [[(K + 1) * NHR * BD, NHO], [BD, NHR],
                         [1, BD], [0, BD]]),
                ALU.mult)
            sn = smpool.tile([NHO, NHR * BD], F32, tag="sn", name=f"sn{c}")
            nc.vector.tensor_reduce(
                bass.AP(sn.tensor, sn[:].offset,
                        [[NHR * BD, NHO], [BD, NHR], [1, BD]]),
                bass.AP(mB.tensor, mB[:].offset,
                        [[NHR * BD * BD, NHO], [BD * BD, NHR],
                         [1, BD], [BD, BD]]),
                axis=mybir.AxisListType.X, op=ALU.add)
            nc.vector.tensor_tensor(
                bass.AP(sg.tensor, sg[:].offset + (c + 1) * NHR * BD,
                        [[(K + 1) * NHR * BD, NHO], [1, NHR * BD]]),
                bass.AP(sn.tensor, sn[:].offset,
                        [[NHR * BD, NHO], [1, NHR * BD]]),
                bass.AP(tub.tensor, tub[:].offset + c * STW + BD * BD,
                        [[K * STW, NHO], [JW * BD, NHR], [1, BD]]),
                ALU.add)

        # scatter sigma_c -> sc[(c,ho), (hr,k)], then expand over i
        sc_t = scpool.tile([P, NHR * BD], F16, tag="sc")
        for ho in range(NHO):
            nc.sync.dma_start(
                sc_t[ho::NHO, :],
                bass.AP(sg.tensor, sg[:].offset + ho * (K + 1) * NHR * BD,
                        [[(K + 1) * NHR * BD, 1], [NHR * BD, K],
                         [1, NHR * BD]]))
        sexp = scpool.tile([P, NHR * BD * BD], F16, tag="sexp")
        nc.vector.tensor_copy(
            bass.AP(sexp.tensor, sexp[:].offset,
                    [[NHR * BD * BD, P], [BD, NHR * BD], [1, BD]]),
            bass.AP(sc_t.tensor, sc_t[:].offset,
                    [[NHR * BD, P], [1, NHR * BD], [0, BD]]))

        # ================= phase C =================
        NTD = NT - 4  # DVE handles tau 0..11, Pool 12..15
        s_outA = scpool.tile([P, NTD * BD * NHR * BD], F32, tag="soutA")
        s_outB = scpool.tile([P, 4 * BD * NHR * BD], F32, tag="soutB")
        PCW = BD * NHR * BD * BD  # 2048
        for tau in range(NT):
            on_pool = (tau >= NTD)
            eng = nc.gpsimd if on_pool else nc.vector
            if tau >= NT - 2 and f'tu8_{tau}' in tu_box:
                tuc = tu_box[f'tu8_{tau}']  # still live in SBUF
            else:
                tpool = tcpoolB if on_pool else tcpool
                tuc = tpool.tile([P, BD, STW], F16, tag="tuc",
                                 name=f"tuc{tau}")
                nc.sync.dma_start(tuc[:], bass.AP(
                    tu_dram, tau * P * BD * STW,
                    [[BD * STW, P], [1, BD * STW]]))
            if on_pool:
                m = pcpoolB.tile([P, PCW], F16, tag="pcm", name=f"pcm{tau}")
                m1 = pcpoolB.tile([P, PCW // 2], F16, tag="pcm1",
                                  name=f"pcm1{tau}")
                m2 = pcpoolB.tile([P, PCW // 4], F16, tag="pcm2",
                                  name=f"pcm2{tau}")
                m3 = pcpoolB.tile([P, PCW // 8], F16, tag="pcm3",
                                  name=f"pcm3{tau}")
            else:
                m = mopool.tile([P, NHR * JW * BD * BD], F16, tag="mo",
                                name=f"pcm{tau}")
                m1 = mopool.tile([P, NHR * JW * BD * 4], F16, tag="m1",
                                 name=f"pcm1{tau}")
                m2 = mopool.tile([P, NHR * JW * BD * 2], F16, tag="m2",
                                 name=f"pcm2{tau}")
                m3 = mopool.tile([P, NHR * JW * BD * BD], F16, tag="mo",
                                 name=f"pcm3{tau}")
            MP = m[:].ap[0][0]
            M1P = m1[:].ap[0][0]
            M2P = m2[:].ap[0][0]
            M3P = m3[:].ap[0][0]
            # m[jr, hr, (k,i)] = ST[jr,hr,jcol=k,i] * sigma_exp[hr,(k,i)]
            eng.tensor_tensor(
                bass.AP(m.tensor, m[:].offset,
                        [[MP, P], [NHR * BD * BD, BD],
                         [BD * BD, NHR], [1, BD * BD]]),
                bass.AP(tuc.tensor, tuc[:].offset,
                        [[BD * STW, P], [STW, BD], [JW * BD, NHR],
                         [1, BD * BD]]),
                bass.AP(sexp.tensor, sexp[:].offset,
                        [[NHR * BD * BD, P], [0, BD], [BD * BD, NHR],
                         [1, BD * BD]]),
                ALU.mult)
            eng.tensor_tensor(
                bass.AP(m1.tensor, m1[:].offset, [[M1P, P], [1, PCW // 2]]),
                bass.AP(m.tensor, m[:].offset,
                        [[MP, P], [BD * BD, BD * NHR], [1, 4 * BD]]),
                bass.AP(m.tensor, m[:].offset + 4 * BD,
                        [[MP, P], [BD * BD, BD * NHR], [1, 4 * BD]]),
                ALU.add)
            eng.tensor_tensor(
                bass.AP(m2.tensor, m2[:].offset, [[M2P, P], [1, PCW // 4]]),
                bass.AP(m1.tensor, m1[:].offset,
                        [[M1P, P], [4 * BD, BD * NHR], [1, 2 * BD]]),
                bass.AP(m1.tensor, m1[:].offset + 2 * BD,
                        [[M1P, P], [4 * BD, BD * NHR], [1, 2 * BD]]),
                ALU.add)
            eng.tensor_tensor(
                bass.AP(m3.tensor, m3[:].offset, [[M3P, P], [1, PCW // 8]]),
                bass.AP(m2.tensor, m2[:].offset,
                        [[M2P, P], [2 * BD, BD * NHR], [1, BD]]),
                bass.AP(m2.tensor, m2[:].offset + BD,
                        [[M2P, P], [2 * BD, BD * NHR], [1, BD]]),
                ALU.add)
            # s = m3 + u  -> s_out half (f32)
            s_o = s_outB if on_pool else s_outA
            s_off = (tau - NTD if on_pool else tau) * BD * NHR * BD
            eng.tensor_tensor(
                bass.AP(s_o.tensor, s_o[:].offset + s_off,
                        [[s_o[:].ap[0][0], P], [1, BD * NHR * BD]]),
                bass.AP(m3.tensor, m3[:].offset, [[M3P, P], [1, PCW // 8]]),
                bass.AP(tuc.tensor, tuc[:].offset + BD * BD,
                        [[BD * STW, P], [STW, BD], [JW * BD, NHR], [1, BD]]),
                ALU.add)

        # ---- output: s_out halves [(c,ho), (pos, hr, i)] -> out[t, vf] ----
        for c in range(K):
            nc.sync.dma_start(
                bass.AP(out, c * CHK * VF,
                        [[NHR * BD, NHO], [VF, NTD * BD], [1, NHR * BD]]),
                bass.AP(s_outA.tensor,
                        s_outA[c * NHO:(c + 1) * NHO, :].offset,
                        [[NTD * BD * NHR * BD, NHO], [NHR * BD, NTD * BD],
                         [1, NHR * BD]]))
            nc.scalar.dma_start(
                bass.AP(out, (c * CHK + NTD * BD) * VF,
                        [[NHR * BD, NHO], [VF, 4 * BD], [1, NHR * BD]]),
                bass.AP(s_outB.tensor,
                        s_outB[c * NHO:(c + 1) * NHO, :].offset,
                        [[4 * BD * NHR * BD, NHO], [NHR * BD, 4 * BD],
                         [1, NHR * BD]]))

    nc.compile()
    return nc


# ---------------- host side ----------------

_NC_CACHE = {}


def _get_nc(TOK=SEQ):
    if TOK not in _NC_CACHE:
        _NC_CACHE[TOK] = build_nc()
    return _NC_CACHE[TOK]


def prep_shared(W1, b1, W2, b2, V1, c1, V2, c2, a0):
    bf = ml_dtypes.bfloat16
    W2r = np.asarray(W2).reshape(H, BD, BD, HID)
    W2c = (W2r - W2r.mean(axis=1, keepdims=True)).reshape(H * BD * BD, HID)
    b2r = np.asarray(b2).reshape(H, BD, BD)
    b2c = (b2r - b2r.mean(axis=1, keepdims=True)).reshape(-1)
    shared = {
        "w1": np.ascontiguousarray(np.asarray(W1).T).astype(bf),
        "b1": np.asarray(b1).reshape(HID, 1).astype(np.float32),
        "v1": np.ascontiguousarray(np.asarray(V1).T).astype(bf),
        "c1": np.asarray(c1).reshape(EMB, 1).astype(np.float32),
    }
    halves = []
    for half in range(2):
        rsl = slice(half * NF, (half + 1) * NF)
        vsl = slice(half * VF, (half + 1) * VF)
        hsl = slice(half * HL, (half + 1) * HL)
        a0h = np.asarray(a0)[0, hsl]                       # [32, 8]
        a0p = a0h.reshape(NHO, NHR * BD)                   # [8, 32]
        halves.append({
            "w2": np.ascontiguousarray(W2c[rsl].T).astype(bf),
            "b2": b2c[rsl].reshape(1, NF).astype(bf),
            "v2": np.ascontiguousarray(np.asarray(V2)[vsl].T).astype(bf),
            "c2": np.asarray(c2)[vsl].reshape(1, VF).astype(bf),
            "a0": a0p.astype(np.float16),
        })
    return shared, halves


def make_in_maps(x, W1, b1, W2, b2, V1, c1, V2, c2, a0):
    shared, halves = prep_shared(W1, b1, W2, b2, V1, c1, V2, c2, a0)
    bf = ml_dtypes.bfloat16
    in_maps = []
    for core in range(N_CORES):
        b, half = core // 2, core % 2
        m = dict(shared)
        m.update(halves[half])
        m["xT"] = np.ascontiguousarray(np.asarray(x)[b].T).astype(bf)
        in_maps.append(m)
    return in_maps


def kernel(x, W1, b1, W2, b2, V1, c1, V2, c2, a0):
    from concourse import bass_utils
    nc = _get_nc(SEQ)
    in_maps = make_in_maps(x, W1, b1, W2, b2, V1, c1, V2, c2, a0)
    res = bass_utils.run_bass_kernel_spmd(nc, in_maps,
                                          core_ids=list(range(N_CORES)))
    out = np.zeros((BS, SEQ, EMB), np.float32)
    for core in range(N_CORES):
        b, half = core // 2, core % 2
        out[b, :, half * VF:(half + 1) * VF] = res.results[core]["out"]
    return out


# revision 5
# speedup vs baseline: 1.5874x; 1.0040x over previous
"""Trainium2 Bass kernel v2 for nn_BlockModel_82678120448388.

Per core (batch b, head-half): MLPs on PE (bf16, W1 resident, batched W2
stream), p=1.2 norm per 512-feat slice (Act + Pool reduce, one act table),
chunked scan (K=16 chunks x 128 steps) on DVE in fp16 with 2x_1p layouts:
state ST[jcol,i] = [T|u]^T updated per step via mult + k-tree adds.
Per-step [T|u] stored to DRAM (fp16); phase C is bulk-parallel
s_t = u_t + T_t sigma_c (split DVE/Pool, no serial tail).
"""

import numpy as np
import ml_dtypes
from contextlib import ExitStack

import concourse.bass as bass
import concourse.bacc as bacc
import concourse.tile as tile
from concourse import mybir

F32 = mybir.dt.float32
F16 = mybir.dt.float16
BF16 = mybir.dt.bfloat16
AF = mybir.ActivationFunctionType
ALU = mybir.AluOpType

BS, SEQ, EMB, BD = 4, 2048, 512, 8
H = EMB // BD
HL = 32            # heads per core
NF = HL * BD * BD  # 2048 blk feats per core
VF = HL * BD       # 256 v feats per core
HID = EMB * BD     # 4096
P = 128
K = 16             # chunks
NHO = P // K       # 8 head-groups on partitions
NHR = HL // NHO    # 4 heads per group (free dim)
JW = BD + 1        # 9 jcols of ST ([T|u]^T)
STW = NHR * JW * BD  # 288 = per-partition ST row
NT = SEQ // P      # 16 token tiles
QT = 512
NQ = SEQ // QT     # 4
TPQ = QT // P      # 4
NSL = 4            # 512-wide feat slices
SL = NF // NSL     # 512
HSL = HL // NSL    # 8 heads per slice
CHK = SEQ // K     # 128 positions per chunk

N_CORES = 8
ACT_SET = 6        # natural_log_exp_and_others: ln/exp/relu/square/copy

# tuning knobs
MULT_SPLIT_HR = False   # True: 4 smaller 4-dim mults instead of one 5-dim
PC_POOL_MOD = 4         # phase-C tiles with tau % PC_POOL_MOD == 3 go to Pool


def build_nc():
    nc = bacc.Bacc("TRN2", target_bir_lowering=False, debug=False)

    xT = nc.dram_tensor("xT", [EMB, SEQ], BF16, kind="ExternalInput")
    w1 = nc.dram_tensor("w1", [EMB, HID], BF16, kind="ExternalInput")
    b1 = nc.dram_tensor("b1", [HID, 1], F32, kind="ExternalInput")
    w2 = nc.dram_tensor("w2", [HID, NF], BF16, kind="ExternalInput")
    b2 = nc.dram_tensor("b2", [1, NF], BF16, kind="ExternalInput")
    v1 = nc.dram_tensor("v1", [EMB, EMB], BF16, kind="ExternalInput")
    c1 = nc.dram_tensor("c1", [EMB, 1], F32, kind="ExternalInput")
    v2 = nc.dram_tensor("v2", [EMB, VF], BF16, kind="ExternalInput")
    c2 = nc.dram_tensor("c2", [1, VF], BF16, kind="ExternalInput")
    a0 = nc.dram_tensor("a0", [NHO, NHR * BD], F16, kind="ExternalInput")
    out = nc.dram_tensor("out", [SEQ, VF], F32, kind="ExternalOutput")

    # scratch in scan order: [tau][c][ho][jr][payload]
    a_dram = nc.dram_tensor("a_scratch", [NT * P * NF], F16)
    v_dram = nc.dram_tensor("v_scratch", [NT * P * VF], F16)
    tu_dram = nc.dram_tensor("tu_scratch", [NT * P * BD * STW], F16)

    with ExitStack() as ctx:
        tc = ctx.enter_context(tile.TileContext(nc))
        cpool = ctx.enter_context(tc.tile_pool(name="consts", bufs=1))
        wpool = ctx.enter_context(tc.tile_pool(name="weights", bufs=1))
        xpool = ctx.enter_context(tc.tile_pool(name="xstream", bufs=2))
        hpool = ctx.enter_context(tc.tile_pool(name="hidden", bufs=1))
        hvpool = ctx.enter_context(tc.tile_pool(name="hv", bufs=1))
        w2pool = ctx.enter_context(tc.tile_pool(name="w2stream", bufs=3))
        l1ps = ctx.enter_context(tc.tile_pool(name="l1ps", bufs=3, space="PSUM"))
        l2ps = ctx.enter_context(tc.tile_pool(name="l2ps", bufs=TPQ, space="PSUM"))
        vps = ctx.enter_context(tc.tile_pool(name="vps", bufs=1, space="PSUM"))
        blkpool = ctx.enter_context(tc.tile_pool(name="blk", bufs=4))
        pwpool = ctx.enter_context(tc.tile_pool(name="pw", bufs=2))
        smpool = ctx.enter_context(tc.tile_pool(name="small", bufs=4))
        aipool = ctx.enter_context(tc.tile_pool(name="ai", bufs=3))
        vtpool = ctx.enter_context(tc.tile_pool(name="vtile", bufs=2))
        agpool = ctx.enter_context(tc.tile_pool(name="ag", bufs=2))
        vgpool = ctx.enter_context(tc.tile_pool(name="vg", bufs=2))
        tupool = ctx.enter_context(tc.tile_pool(name="tu", bufs=2))
        mopool = ctx.enter_context(tc.tile_pool(name="mo", bufs=2))
        scpool = ctx.enter_context(tc.tile_pool(name="scan", bufs=1))
        tcpool = ctx.enter_context(tc.tile_pool(name="tuc", bufs=3))
        tcpoolB = ctx.enter_context(tc.tile_pool(name="tucB", bufs=1))
        pcpoolB = ctx.enter_context(tc.tile_pool(name="pcB", bufs=1))

        nc.scalar.add_instruction(mybir.InstLoadActFuncSet(
            name=nc.get_next_instruction_name(), act_func_set_id=ACT_SET,
            ins=[], outs=[]))

        # ---- constants / weights ----
        ones_s = cpool.tile([1, P], BF16, tag="ones")
        nc.vector.memset(ones_s[:], 1.0)
        w1_s = wpool.tile([P, 4, HID], BF16, tag="w1")
        for k in range(4):
            nc.sync.dma_start(
                w1_s[:, k, bass.ds(0, HID // 4)],
                bass.AP(w1, k * P * HID,
                        [[HID, P], [1, HID // 4]]))
        xq0 = xpool.tile([P, 4, QT], BF16, tag="xq", name="xq_pre0")
        for k in range(4):
            for ti in range(4):
                nc.sync.dma_start(
                    xq0[:, k, bass.ts(ti, P)],
                    bass.AP(xT, k * P * SEQ + ti * 8,
                            [[SEQ, P], [CHK, K], [1, 8]]))
        b1_s = cpool.tile([P, HID // P], F32, tag="b1")
        nc.sync.dma_start(b1_s[:], b1[:].rearrange("(m p) one -> p (m one)", p=P))
        c1_s = cpool.tile([P, EMB // P], F32, tag="c1")
        nc.sync.dma_start(c1_s[:], c1[:].rearrange("(m p) one -> p (m one)", p=P))
        b2_s = cpool.tile([1, NF], BF16, tag="b2")
        nc.sync.dma_start(b2_s[:], b2[:])
        c2_s = cpool.tile([1, VF], BF16, tag="c2")
        nc.sync.dma_start(c2_s[:], c2[:])
        a0_s = cpool.tile([NHO, NHR * BD], F16, tag="a0")
        nc.sync.dma_start(a0_s[:], a0[:])
        for k in range(4):
            for mg in range(1, 4):
                nc.sync.dma_start(
                    w1_s[:, k, bass.ds(mg * HID // 4, HID // 4)],
                    bass.AP(w1, k * P * HID + mg * HID // 4,
                            [[HID, P], [1, HID // 4]]))
        v1_s = wpool.tile([P, 4, EMB], BF16, tag="v1")
        nc.sync.dma_start(v1_s[:], v1[:].rearrange("(k p) m -> p k m", p=P))
        v2_s = wpool.tile([P, 4, VF], BF16, tag="v2")
        nc.sync.dma_start(v2_s[:], v2[:].rearrange("(k p) n -> p k n", p=P))

        # ================= phase-1 =================
        # per partition (c,ho), per hr: ST[jcol, i] fp16 (jcol 9, i 8)
        # step: ST_new[jcol,i] = sum_k ST[jcol,k]*A[i,k]; u row += v
        tu_box = {}

        def p1_step(tau, jr, a8, v8, tu8):
            if tau == 0 and jr == 0:
                nc.vector.tensor_copy(
                    bass.AP(tu8.tensor, tu8[:].offset,
                            [[BD * STW, P], [JW * BD, NHR], [1, BD], [BD, BD]]),
                    bass.AP(a8.tensor, a8[:].offset,
                            [[BD * NHR * BD * BD, P], [BD * BD, NHR],
                             [BD, BD], [1, BD]]))
                nc.vector.tensor_copy(
                    bass.AP(tu8.tensor, tu8[:].offset + BD * BD,
                            [[BD * STW, P], [JW * BD, NHR], [1, BD]]),
                    bass.AP(v8.tensor, v8[:].offset,
                            [[BD * NHR * BD, P], [BD, NHR], [1, BD]]))
                return
            if jr == 0:
                src_t, src_off = tu_box['prev'], (BD - 1) * STW
            else:
                src_t, src_off = tu8, (jr - 1) * STW
            mo = mopool.tile([P, NHR * JW * BD * BD], F16, tag="mo",
                             name=f"mo{tau}_{jr}")
            # mo[hr, jcol, i, k] = ST[hr, jcol, k] * A[hr, i, k]
            if MULT_SPLIT_HR:
                for hr in range(NHR):
                    nc.vector.tensor_tensor(
                        bass.AP(mo.tensor, mo[:].offset + hr * JW * BD * BD,
                                [[NHR * JW * BD * BD, P], [BD * BD, JW],
                                 [BD, BD], [1, BD]]),
                        bass.AP(src_t.tensor,
                                src_t[:].offset + src_off + hr * JW * BD,
                                [[BD * STW, P], [BD, JW], [0, BD], [1, BD]]),
                        bass.AP(a8.tensor,
                                a8[:].offset + jr * NHR * BD * BD + hr * BD * BD,
                                [[BD * NHR * BD * BD, P], [0, JW],
                                 [BD, BD], [1, BD]]),
                        ALU.mult)
            else:
                nc.vector.tensor_tensor(
                    bass.AP(mo.tensor, mo[:].offset,
                            [[NHR * JW * BD * BD, P], [JW * BD * BD, NHR],
                             [BD * BD, JW], [BD, BD], [1, BD]]),
                    bass.AP(src_t.tensor, src_t[:].offset + src_off,
                            [[BD * STW, P], [JW * BD, NHR],
                             [BD, JW], [0, BD], [1, BD]]),
                    bass.AP(a8.tensor, a8[:].offset + jr * NHR * BD * BD,
                            [[BD * NHR * BD * BD, P], [BD * BD, NHR],
                             [0, JW], [BD, BD], [1, BD]]),
                    ALU.mult)
            m1 = mopool.tile([P, NHR * JW * BD * 4], F16, tag="m1",
                             name=f"m1{tau}_{jr}")
            nc.vector.tensor_tensor(
                m1[:],
                bass.AP(mo.tensor, mo[:].offset,
                        [[NHR * JW * BD * BD, P], [BD, NHR * JW * BD], [1, 4]]),
                bass.AP(mo.tensor, mo[:].offset + 4,
                        [[NHR * JW * BD * BD, P], [BD, NHR * JW * BD], [1, 4]]),
                ALU.add)
            m2 = mopool.tile([P, NHR * JW * BD * 2], F16, tag="m2",
                             name=f"m2{tau}_{jr}")
            nc.vector.tensor_tensor(
                m2[:],
                bass.AP(m1.tensor, m1[:].offset,
                        [[NHR * JW * BD * 4, P], [4, NHR * JW * BD], [1, 2]]),
                bass.AP(m1.tensor, m1[:].offset + 2,
                        [[NHR * JW * BD * 4, P], [4, NHR * JW * BD], [1, 2]]),
                ALU.add)
            nc.vector.tensor_tensor(
                bass.AP(tu8.tensor, tu8[:].offset + jr * STW,
                        [[BD * STW, P], [1, STW]]),
                bass.AP(m2.tensor, m2[:].offset,
                        [[NHR * JW * BD * 2, P], [2, STW]]),
                bass.AP(m2.tensor, m2[:].offset + 1,
                        [[NHR * JW * BD * 2, P], [2, STW]]),
                ALU.add)
            urow = bass.AP(tu8.tensor, tu8[:].offset + jr * STW + BD * BD,
                           [[BD * STW, P], [JW * BD, NHR], [1, BD]])
            nc.vector.tensor_tensor(
                urow, urow,
                bass.AP(v8.tensor, v8[:].offset + jr * NHR * BD,
                        [[BD * NHR * BD, P], [BD, NHR], [1, BD]]),
                ALU.add)

        def p1_tile(tau):
            if tau >= NT - 2:
                tu_box[f'tu8_{tau}'] = None  # filled below
            a8 = agpool.tile([P, BD, NHR * BD * BD], F16, tag="a8",
                             name=f"a8_{tau}")
            v8 = vgpool.tile([P, BD, NHR * BD], F16, tag="v8", name=f"v8_{tau}")
            geng = nc.gpsimd
            geng.dma_start(a8[:], bass.AP(
                a_dram, tau * P * NF, [[BD * 256, P], [1, BD * 256]]))
            geng.dma_start(v8[:], bass.AP(
                v_dram, tau * P * VF, [[BD * 32, P], [1, BD * 32]]))
            tu8 = tupool.tile([P, BD, STW], F16, tag="tu8", name=f"tu8_{tau}")
            for jr in range(BD):
                p1_step(tau, jr, a8, v8, tu8)
            tu_box['prev'] = tu8
            tu_box[f'tu8_{tau}'] = tu8
            nc.scalar.dma_start(
                bass.AP(tu_dram, tau * P * BD * STW,
                        [[BD * STW, P], [1, BD * STW]]),
                tu8[:])

        # ================= stage A =================
        SCHED = [4, 4, 4, 2, 2]  # tiles per q-group (tapered tail)
        t0 = 0
        for qi, nt_q in enumerate(SCHED):
            W = nt_q * P
            if qi == 0:
                xq = xq0
            else:
                xq = xpool.tile([P, 4, QT], BF16, tag="xq", name=f"xq{qi}")
                for ti in range(nt_q):
                    for k in range(4):
                        nc.sync.dma_start(
                            xq[:, k, bass.ts(ti, P)],
                            bass.AP(xT, k * P * SEQ + (t0 + ti) * 8,
                                    [[SEQ, P], [CHK, K], [1, 8]]))

            hid_t = hpool.tile([P, HID // P, QT], BF16, tag="hid",
                               name=f"hid{qi}")
            for m in range(HID // P):
                ps = l1ps.tile([P, QT], F32, tag="l1")
                for k in range(4):
                    nc.tensor.matmul(ps[:, :W], w1_s[:, k, bass.ts(m, P)],
                                     xq[:, k, :W], start=(k == 0),
                                     stop=(k == 3))
                nc.scalar.activation(hid_t[:, m, :W], ps[:, :W], AF.Relu,
                                     bias=b1_s[:, m:m + 1])

            hv_t = hvpool.tile([P, 4, QT], BF16, tag="hv", name=f"hv{qi}")
            for m in range(4):
                ps = l1ps.tile([P, QT], F32, tag="l1")
                for k in range(4):
                    nc.tensor.matmul(ps[:, :W], v1_s[:, k, bass.ts(m, P)],
                                     xq[:, k, :W], start=(k == 0),
                                     stop=(k == 3))
                nc.scalar.activation(hv_t[:, m, :W], ps[:, :W], AF.Relu,
                                     bias=c1_s[:, m:m + 1])

            # ---- L2 + per-slice norm ----
            for n in range(NSL):
                pss = [l2ps.tile([P, SL], F32, tag="l2",
                                 name=f"l2_{qi}_{n}_{i}")
                       for i in range(nt_q)]
                for ti in range(nt_q):
                    nc.tensor.matmul(pss[ti][:], ones_s[:1, :],
                                     b2_s[:1, bass.ts(n, SL)],
                                     start=True, stop=False)
                for kg in range(8):
                    w2s = w2pool.tile([P, 4, SL], BF16, tag="w2s",
                                      name=f"w2_{qi}_{n}_{kg}")
                    nc.sync.dma_start(w2s[:], bass.AP(
                        w2, kg * 4 * P * NF + n * SL,
                        [[NF, P], [P * NF, 4], [1, SL]]))
                    for k8 in range(4):
                        k = kg * 4 + k8
                        for ti in range(nt_q):
                            nc.tensor.matmul(
                                pss[ti][:], hid_t[:, k, bass.ts(ti, P)],
                                w2s[:, k8, :], start=False,
                                stop=(k == HID // P - 1))
                # psum-freeing copies first, then the norm chains
                blks = []
                for ti in range(nt_q):
                    blk = blkpool.tile([P, SL], F32, tag="blk",
                                       name=f"blk{qi}_{n}_{ti}")
                    nc.scalar.activation(blk[:], pss[ti][:], AF.Identity)
                    blks.append(blk)
                for ti in range(nt_q):
                    blk = blks[ti]
                    pw = pwpool.tile([P, SL], F32, tag="pw")
                    nc.scalar.activation(pw[:], blk[:], AF.Square)
                    nc.scalar.activation(pw[:], pw[:], AF.Ln)
                    nc.scalar.activation(pw[:], pw[:], AF.Exp, scale=0.6)
                    pst = smpool.tile([P, HSL * BD], F32, tag="pst")
                    nc.vector.tensor_reduce(
                        pst[:].rearrange("p (h j) -> p h j", h=HSL, j=BD),
                        bass.AP(pw.tensor, pw[:].offset,
                                [[SL, P], [64, HSL], [1, BD], [8, BD]]),
                        axis=mybir.AxisListType.X, op=ALU.add)
                    dm = smpool.tile([P, HSL], F32, tag="dm")
                    nc.vector.tensor_reduce(
                        dm[:].rearrange("p (h one) -> p h one", h=HSL, one=1),
                        pst[:].rearrange("p (h j) -> p h j", h=HSL, j=BD),
                        axis=mybir.AxisListType.X, op=ALU.max)
                    rc = smpool.tile([P, HSL], F32, tag="rc")
                    nc.scalar.activation(rc[:], dm[:], AF.Ln)
                    nc.scalar.activation(rc[:], rc[:], AF.Exp,
                                         scale=-1.0 / 1.2)
                    asl = aipool.tile([P, SL], F16, tag="ai",
                                      name=f"ai{qi}_{n}_{ti}")
                    nc.gpsimd.tensor_tensor(
                        asl[:].rearrange("p (h i k) -> p h i k",
                                         h=HSL, i=BD, k=BD),
                        blk[:].rearrange("p (h i k) -> p h i k",
                                         h=HSL, i=BD, k=BD),
                        bass.AP(rc.tensor, rc[:].offset,
                                [[HSL, P], [1, HSL], [0, BD], [0, BD]]),
                        ALU.mult)
                    tt = t0 + ti
                    weng = nc.gpsimd
                    for ho2 in range(2):
                        # dst (tau, c, ho=2n+ho2, jr=j, (hr,i,k))
                        weng.dma_start(
                            bass.AP(a_dram,
                                    tt * P * NF + (2 * n + ho2) * BD * 256,
                                    [[NHO * BD * 256, K], [256, BD],
                                     [1, 256]]),
                            bass.AP(asl.tensor, asl[:].offset + ho2 * 256,
                                    [[SL, P], [1, 256]]))

            for ti in range(nt_q):
                tt = t0 + ti
                psv = vps.tile([P, VF], F32, tag="v")
                nc.tensor.matmul(psv[:], ones_s[:1, :], c2_s[:1, :],
                                 start=True, stop=False)
                for k in range(4):
                    nc.tensor.matmul(psv[:], hv_t[:, k, bass.ts(ti, P)],
                                     v2_s[:, k, :], start=False,
                                     stop=(k == 3))
                vt = vtpool.tile([P, VF], F16, tag="vt")
                nc.scalar.activation(vt[:], psv[:], AF.Identity)
                veng = nc.gpsimd
                for ho in range(NHO):
                    veng.dma_start(
                        bass.AP(v_dram, tt * P * VF + ho * BD * 32,
                                [[NHO * BD * 32, K], [32, BD], [1, 32]]),
                        bass.AP(vt.tensor, vt[:].offset + ho * 32,
                                [[VF, P], [1, 32]]))

                p1_tile(tt)
            t0 += nt_q

        # ================= phase B =================
        # extract per-chunk summaries from tu8 of the last tile (SBUF->SBUF)
        tu_last = tu_box['prev']
        tub = scpool.tile([NHO, K * STW], F16, tag="tub")
        for c in range(K):
            qeng = nc.sync if c % 2 == 0 else nc.scalar
            qeng.dma_start(
                bass.AP(tub.tensor, tub[:].offset + c * STW,
                        [[K * STW, NHO], [1, STW]]),
                bass.AP(tu_last.tensor,
                        tu_last[:].offset + c * NHO * BD * STW
                        + (BD - 1) * STW,
                        [[BD * STW, NHO], [1, STW]]))
        sg = scpool.tile([NHO, (K + 1) * NHR * BD], F16, tag="sg")
        nc.vector.tensor_copy(sg[:, 0:NHR * BD], a0_s[:])
        for c in range(K):
            mB = smpool.tile([NHO, NHR * BD * BD], F32, tag="mB", name=f"mB{c}")
            nc.vector.tensor_tensor(
                bass.AP(mB.tensor, mB[:].offset,
                        [[NHR * BD * BD, NHO], [BD * BD, NHR],
                         [BD, BD], [1, BD]]),
                bass.AP(tub.tensor, tub[:].offset + c * STW,
                        [[K * STW, NHO], [JW * BD, NHR], [BD, BD], [1, BD]]),
                bass.AP(sg.tensor, sg[:].offset + c * NHR * BD,
                        [[(K + 1) * NHR * BD, NHO], [BD, NHR],
                         [1, BD], [0, BD]]),
                ALU.mult)
            sn = smpool.tile([NHO, NHR * BD], F32, tag="sn", name=f"sn{c}")
            nc.vector.tensor_reduce(
                bass.AP(sn.tensor, sn[:].offset,
                        [[NHR * BD, NHO], [BD, NHR], [1, BD]]),
                bass.AP(mB.tensor, mB[:].offset,
                        [[NHR * BD * BD, NHO], [BD * BD, NHR],
                         [1, BD], [BD, BD]]),
                axis=mybir.AxisListType.X, op=ALU.add)
            nc.vector.tensor_tensor(
                bass.AP(sg.tensor, sg[:].offset + (c + 1) * NHR * BD,
                        [[(K + 1) * NHR * BD, NHO], [1, NHR * BD]]),
                bass.AP(sn.tensor, sn[:].offset,
                        [[NHR * BD, NHO], [1, NHR * BD]]),
                bass.AP(tub.tensor, tub[:].offset + c * STW + BD * BD,
                        [[K * STW, NHO], [JW * BD, NHR], [1, BD]]),
                ALU.add)

        # scatter sigma_c -> sc[(c,ho), (hr,k)], then expand over i
        sc_t = scpool.tile([P, NHR * BD], F16, tag="sc")
        for ho in range(NHO):
            nc.sync.dma_start(
                sc_t[ho::NHO, :],
                bass.AP(sg.tensor, sg[:].offset + ho * (K + 1) * NHR * BD,
                        [[(K + 1) * NHR * BD, 1], [NHR * BD, K],
                         [1, NHR * BD]]))
        sexp = scpool.tile([P, NHR * BD * BD], F16, tag="sexp")
        nc.vector.tensor_copy(
            bass.AP(sexp.tensor, sexp[:].offset,
                    [[NHR * BD * BD, P], [BD, NHR * BD], [1, BD]]),
            bass.AP(sc_t.tensor, sc_t[:].offset,
                    [[NHR * BD, P], [1, NHR * BD], [0, BD]]))

        # ================= phase C =================
        NTD = NT - 4  # DVE handles tau 0..11, Pool 12..15
        s_outA = scpool.tile([P, NTD * BD * NHR * BD], F32, tag="soutA")
        s_outB = scpool.tile([P, 4 * BD * NHR * BD], F32, tag="soutB")
        PCW = BD * NHR * BD * BD  # 2048
        for tau in range(NT):
            on_pool = (tau >= NTD)
            eng = nc.gpsimd if on_pool else nc.vector
            if tau >= NT - 2 and f'tu8_{tau}' in tu_box:
                tuc = tu_box[f'tu8_{tau}']  # still live in SBUF
            else:
                tpool = tcpoolB if on_pool else tcpool
                tuc = tpool.tile([P, BD, STW], F16, tag="tuc",
                                 name=f"tuc{tau}")
                nc.sync.dma_start(tuc[:], bass.AP(
                    tu_dram, tau * P * BD * STW,
                    [[BD * STW, P], [1, BD * STW]]))
            if on_pool:
                m = pcpoolB.tile([P, PCW], F16, tag="pcm", name=f"pcm{tau}")
                m1 = pcpoolB.tile([P, PCW // 2], F16, tag="pcm1",
                                  name=f"pcm1{tau}")
                m2 = pcpoolB.tile([P, PCW // 4], F16, tag="pcm2",
                                  name=f"pcm2{tau}")
                m3 = pcpoolB.tile([P, PCW // 8], F16, tag="pcm3",
                                  name=f"pcm3{tau}")
            else:
                m = mopool.tile([P, NHR * JW * BD * BD], F16, tag="mo",
                                name=f"pcm{tau}")
                m1 = mopool.tile([P, NHR * JW * BD * 4], F16, tag="m1",
                                 name=f"pcm1{tau}")
                m2 = mopool.tile([P, NHR * JW * BD * 2], F16, tag="m2",
                                 name=f"pcm2{tau}")
                m3 = mopool.tile([P, NHR * JW * BD * BD], F16, tag="mo",
                                 name=f"pcm3{tau}")
            MP = m[:].ap[0][0]
            M1P = m1[:].ap[0][0]
            M2P = m2[:].ap[0][0]
            M3P = m3[:].ap[0][0]
            # m[jr, hr, (k,i)] = ST[jr,hr,jcol=k,i] * sigma_exp[hr,(k,i)]
            eng.tensor_tensor(
                bass.AP(m.tensor, m[:].offset,
                        [[MP, P], [NHR * BD * BD, BD],
                         [BD * BD, NHR], [1, BD * BD]]),
                bass.AP(tuc.tensor, tuc[:].offset,
                        [[BD * STW, P], [STW, BD], [JW * BD, NHR],
                         [1, BD * BD]]),
                bass.AP(sexp.tensor, sexp[:].offset,
                        [[NHR * BD * BD, P], [0, BD], [BD * BD, NHR],
                         [1, BD * BD]]),
                ALU.mult)
            eng.tensor_tensor(
                bass.AP(m1.tensor, m1[:].offset, [[M1P, P], [1, PCW // 2]]),
                bass.AP(m.tensor, m[:].offset,
                        [[MP, P], [BD * BD, BD * NHR], [1, 4 * BD]]),
                bass.AP(m.tensor, m[:].offset + 4 * BD,
                        [[MP, P], [BD * BD, BD * NHR], [1, 4 * BD]]),
                ALU.add)
            eng.tensor_tensor(
                bass.AP(m2.tensor, m2[:].offset, [[M2P, P], [1, PCW // 4]]),
                bass.AP(m1.tensor, m1[:].offset,
                        [[M1P, P], [4 * BD, BD * NHR], [1, 2 * BD]]),
                bass.AP(m1.tensor, m1[:].offset + 2 * BD,
                        [[M1P, P], [4 * BD, BD * NHR], [1, 2 * BD]]),
                ALU.add)
            eng.tensor_tensor(
                bass.AP(m3.tensor, m3[:].offset, [[M3P, P], [1, PCW // 8]]),
                bass.AP(m2.tensor, m2[:].offset,
                        [[M2P, P], [2 * BD, BD * NHR], [1, BD]]),
                bass.AP(m2.tensor, m2[:].offset + BD,
                        [[M2P, P], [2 * BD, BD * NHR], [1, BD]]),
                ALU.add)
            # s = m3 + u  -> s_out half (f32)
            s_o = s_outB if on_pool else s_outA
            s_off = (tau - NTD if on_pool else tau) * BD * NHR * BD
            eng.tensor_tensor(
                bass.AP(s_o.tensor, s_o[:].offset + s_off,
                        [[s_o[:].ap[0][0], P], [1, BD * NHR * BD]]),
                bass.AP(m3.tensor, m3[:].offset, [[M3P, P], [1, PCW // 8]]),
                bass.AP(tuc.tensor, tuc[:].offset + BD * BD,
                        [[BD * STW, P], [STW, BD], [JW * BD, NHR], [1, BD]]),
                ALU.add)

        # ---- output: s_out halves [(c,ho), (pos, hr, i)] -> out[t, vf] ----
        for c in range(K):
            nc.sync.dma_start(
                bass.AP(out, c * CHK * VF,
                        [[NHR * BD, NHO], [VF, NTD * BD], [1, NHR * BD]]),
                bass.AP(s_outA.tensor,
                        s_outA[c * NHO:(c + 1) * NHO, :].offset,
                        [[NTD * BD * NHR * BD, NHO], [NHR * BD, NTD * BD],
                         [1, NHR * BD]]))
            nc.scalar.dma_start(
                bass.AP(out, (c * CHK + NTD * BD) * VF,
                        [[NHR * BD, NHO], [VF, 4 * BD], [1, NHR * BD]]),
                bass.AP(s_outB.tensor,
                        s_outB[c * NHO:(c + 1) * NHO, :].offset,
                        [[4 * BD * NHR * BD, NHO], [NHR * BD, 4 * BD],
                         [1, NHR * BD]]))

    nc.compile()
    return nc


# ---------------- host side ----------------

_NC_CACHE = {}


def _get_nc(TOK=SEQ):
    if TOK not in _NC_CACHE:
        _NC_CACHE[TOK] = build_nc()
    return _NC_CACHE[TOK]


def prep_shared(W1, b1, W2, b2, V1, c1, V2, c2, a0):
    bf = ml_dtypes.bfloat16
    W2r = np.asarray(W2).reshape(H, BD, BD, HID)
    W2c = (W2r - W2r.mean(axis=1, keepdims=True)).reshape(H * BD * BD, HID)
    b2r = np.asarray(b2).reshape(H, BD, BD)
    b2c = (b2r - b2r.mean(axis=1, keepdims=True)).reshape(-1)
    shared = {
        "w1": np.ascontiguousarray(np.asarray(W1).T).astype(bf),
        "b1": np.asarray(b1).reshape(HID, 1).astype(np.float32),
        "v1": np.ascontiguousarray(np.asarray(V1).T).astype(bf),
        "c1": np.asarray(c1).reshape(EMB, 1).astype(np.float32),
    }
    halves = []
    for half in range(2):
        rsl = slice(half * NF, (half + 1) * NF)
        vsl = slice(half * VF, (half + 1) * VF)
        hsl = slice(half * HL, (half + 1) * HL)
        a0h = np.asarray(a0)[0, hsl]                       # [32, 8]
        a0p = a0h.reshape(NHO, NHR * BD)                   # [8, 32]
        halves.append({
            "w2": np.ascontiguousarray(W2c[rsl].T).astype(bf),
            "b2": b2c[rsl].reshape(1, NF).astype(bf),
            "v2": np.ascontiguousarray(np.asarray(V2)[vsl].T).astype(bf),
            "c2": np.asarray(c2)[vsl].reshape(1, VF).astype(bf),
            "a0": a0p.astype(np.float16),
        })
    return shared, halves


def make_in_maps(x, W1, b1, W2, b2, V1, c1, V2, c2, a0):
    shared, halves = prep_shared(W1, b1, W2, b2, V1, c1, V2, c2, a0)
    bf = ml_dtypes.bfloat16
    in_maps = []
    for core in range(N_CORES):
        b, half = core // 2, core % 2
        m = dict(shared)
        m.update(halves[half])
        m["xT"] = np.ascontiguousarray(np.asarray(x)[b].T).astype(bf)
        in_maps.append(m)
    return in_maps


def kernel(x, W1, b1, W2, b2, V1, c1, V2, c2, a0):
    from concourse import bass_utils
    nc = _get_nc(SEQ)
    in_maps = make_in_maps(x, W1, b1, W2, b2, V1, c1, V2, c2, a0)
    res = bass_utils.run_bass_kernel_spmd(nc, in_maps,
                                          core_ids=list(range(N_CORES)))
    out = np.zeros((BS, SEQ, EMB), np.float32)
    for core in range(N_CORES):
        b, half = core // 2, core % 2
        out[b, :, half * VF:(half + 1) * VF] = res.results[core]["out"]
    return out


# revision 6
# speedup vs baseline: 1.5940x; 1.0042x over previous
"""Trainium2 Bass kernel v2 for nn_BlockModel_82678120448388.

Per core (batch b, head-half): MLPs on PE (bf16, W1 resident, batched W2
stream), p=1.2 norm per 512-feat slice (Act + Pool reduce, one act table),
chunked scan (K=16 chunks x 128 steps) on DVE in fp16 with 2x_1p layouts:
state ST[jcol,i] = [T|u]^T updated per step via mult + k-tree adds.
Per-step [T|u] stored to DRAM (fp16); phase C is bulk-parallel
s_t = u_t + T_t sigma_c (split DVE/Pool, no serial tail).
"""

import numpy as np
import ml_dtypes
from contextlib import ExitStack

import concourse.bass as bass
import concourse.bacc as bacc
import concourse.tile as tile
from concourse import mybir

F32 = mybir.dt.float32
F16 = mybir.dt.float16
BF16 = mybir.dt.bfloat16
AF = mybir.ActivationFunctionType
ALU = mybir.AluOpType

BS, SEQ, EMB, BD = 4, 2048, 512, 8
H = EMB // BD
HL = 32            # heads per core
NF = HL * BD * BD  # 2048 blk feats per core
VF = HL * BD       # 256 v feats per core
HID = EMB * BD     # 4096
P = 128
K = 16             # chunks
NHO = P // K       # 8 head-groups on partitions
NHR = HL // NHO    # 4 heads per group (free dim)
JW = BD + 1        # 9 jcols of ST ([T|u]^T)
STW = NHR * JW * BD  # 288 = per-partition ST row
NT = SEQ // P      # 16 token tiles
QT = 512
NQ = SEQ // QT     # 4
TPQ = QT // P      # 4
NSL = 4            # 512-wide feat slices
SL = NF // NSL     # 512
HSL = HL // NSL    # 8 heads per slice
CHK = SEQ // K     # 128 positions per chunk

N_CORES = 8
ACT_SET = 6        # natural_log_exp_and_others: ln/exp/relu/square/copy

# tuning knobs
MULT_SPLIT_HR = False   # True: 4 smaller 4-dim mults instead of one 5-dim
PC_POOL_MOD = 4         # phase-C tiles with tau % PC_POOL_MOD == 3 go to Pool


def build_nc():
    nc = bacc.Bacc("TRN2", target_bir_lowering=False, debug=False)

    xT = nc.dram_tensor("xT", [EMB, SEQ], BF16, kind="ExternalInput")
    w1 = nc.dram_tensor("w1", [EMB, HID], BF16, kind="ExternalInput")
    b1 = nc.dram_tensor("b1", [HID, 1], F32, kind="ExternalInput")
    w2 = nc.dram_tensor("w2", [HID, NF], BF16, kind="ExternalInput")
    b2 = nc.dram_tensor("b2", [1, NF], BF16, kind="ExternalInput")
    v1 = nc.dram_tensor("v1", [EMB, EMB], BF16, kind="ExternalInput")
    c1 = nc.dram_tensor("c1", [EMB, 1], F32, kind="ExternalInput")
    v2 = nc.dram_tensor("v2", [EMB, VF], BF16, kind="ExternalInput")
    c2 = nc.dram_tensor("c2", [1, VF], BF16, kind="ExternalInput")
    a0 = nc.dram_tensor("a0", [NHO, NHR * BD], F16, kind="ExternalInput")
    out = nc.dram_tensor("out", [SEQ, VF], F32, kind="ExternalOutput")

    # scratch in scan order: [tau][c][ho][jr][payload]
    a_dram = nc.dram_tensor("a_scratch", [NT * P * NF], F16)
    v_dram = nc.dram_tensor("v_scratch", [NT * P * VF], F16)
    tu_dram = nc.dram_tensor("tu_scratch", [NT * P * BD * STW], F16)

    with ExitStack() as ctx:
        tc = ctx.enter_context(tile.TileContext(nc))
        cpool = ctx.enter_context(tc.tile_pool(name="consts", bufs=1))
        wpool = ctx.enter_context(tc.tile_pool(name="weights", bufs=1))
        xpool = ctx.enter_context(tc.tile_pool(name="xstream", bufs=2))
        hpool = ctx.enter_context(tc.tile_pool(name="hidden", bufs=1))
        hvpool = ctx.enter_context(tc.tile_pool(name="hv", bufs=1))
        w2pool = ctx.enter_context(tc.tile_pool(name="w2stream", bufs=3))
        l1ps = ctx.enter_context(tc.tile_pool(name="l1ps", bufs=3, space="PSUM"))
        l2ps = ctx.enter_context(tc.tile_pool(name="l2ps", bufs=TPQ, space="PSUM"))
        vps = ctx.enter_context(tc.tile_pool(name="vps", bufs=1, space="PSUM"))
        blkpool = ctx.enter_context(tc.tile_pool(name="blk", bufs=4))
        pwpool = ctx.enter_context(tc.tile_pool(name="pw", bufs=2))
        smpool = ctx.enter_context(tc.tile_pool(name="small", bufs=4))
        aipool = ctx.enter_context(tc.tile_pool(name="ai", bufs=3))
        vtpool = ctx.enter_context(tc.tile_pool(name="vtile", bufs=2))
        agpool = ctx.enter_context(tc.tile_pool(name="ag", bufs=2))
        vgpool = ctx.enter_context(tc.tile_pool(name="vg", bufs=2))
        tupool = ctx.enter_context(tc.tile_pool(name="tu", bufs=2))
        mopool = ctx.enter_context(tc.tile_pool(name="mo", bufs=2))
        scpool = ctx.enter_context(tc.tile_pool(name="scan", bufs=1))
        tcpool = ctx.enter_context(tc.tile_pool(name="tuc", bufs=3))
        tcpoolB = ctx.enter_context(tc.tile_pool(name="tucB", bufs=1))
        pcpoolB = ctx.enter_context(tc.tile_pool(name="pcB", bufs=1))

        nc.scalar.add_instruction(mybir.InstLoadActFuncSet(
            name=nc.get_next_instruction_name(), act_func_set_id=ACT_SET,
            ins=[], outs=[]))

        # ---- constants / weights ----
        ones_s = cpool.tile([1, P], BF16, tag="ones")
        nc.vector.memset(ones_s[:], 1.0)
        w1_s = wpool.tile([P, 4, HID], BF16, tag="w1")
        for k in range(4):
            nc.sync.dma_start(
                w1_s[:, k, bass.ds(0, HID // 4)],
                bass.AP(w1, k * P * HID,
                        [[HID, P], [1, HID // 4]]))
        xq0 = xpool.tile([P, 4, QT], BF16, tag="xq", name="xq_pre0")
        for k in range(4):
            for ti in range(4):
                nc.sync.dma_start(
                    xq0[:, k, bass.ts(ti, P)],
                    bass.AP(xT, k * P * SEQ + ti * 8,
                            [[SEQ, P], [CHK, K], [1, 8]]))
        b1_s = cpool.tile([P, HID // P], F32, tag="b1")
        nc.sync.dma_start(b1_s[:], b1[:].rearrange("(m p) one -> p (m one)", p=P))
        c1_s = cpool.tile([P, EMB // P], F32, tag="c1")
        nc.sync.dma_start(c1_s[:], c1[:].rearrange("(m p) one -> p (m one)", p=P))
        b2_s = cpool.tile([1, NF], BF16, tag="b2")
        nc.sync.dma_start(b2_s[:], b2[:])
        c2_s = cpool.tile([1, VF], BF16, tag="c2")
        nc.sync.dma_start(c2_s[:], c2[:])
        a0_s = cpool.tile([NHO, NHR * BD], F16, tag="a0")
        nc.sync.dma_start(a0_s[:], a0[:])
        for k in range(4):
            for mg in range(1, 4):
                nc.sync.dma_start(
                    w1_s[:, k, bass.ds(mg * HID // 4, HID // 4)],
                    bass.AP(w1, k * P * HID + mg * HID // 4,
                            [[HID, P], [1, HID // 4]]))
        v1_s = wpool.tile([P, 4, EMB], BF16, tag="v1")
        nc.sync.dma_start(v1_s[:], v1[:].rearrange("(k p) m -> p k m", p=P))
        v2_s = wpool.tile([P, 4, VF], BF16, tag="v2")
        nc.sync.dma_start(v2_s[:], v2[:].rearrange("(k p) n -> p k n", p=P))

        # ================= phase-1 =================
        # per partition (c,ho), per hr: ST[jcol, i] fp16 (jcol 9, i 8)
        # step: ST_new[jcol,i] = sum_k ST[jcol,k]*A[i,k]; u row += v
        tu_box = {}

        def p1_step(tau, jr, a8, v8, tu8):
            if tau == 0 and jr == 0:
                nc.vector.tensor_copy(
                    bass.AP(tu8.tensor, tu8[:].offset,
                            [[BD * STW, P], [JW * BD, NHR], [1, BD], [BD, BD]]),
                    bass.AP(a8.tensor, a8[:].offset,
                            [[BD * NHR * BD * BD, P], [BD * BD, NHR],
                             [BD, BD], [1, BD]]))
                nc.vector.tensor_copy(
                    bass.AP(tu8.tensor, tu8[:].offset + BD * BD,
                            [[BD * STW, P], [JW * BD, NHR], [1, BD]]),
                    bass.AP(v8.tensor, v8[:].offset,
                            [[BD * NHR * BD, P], [BD, NHR], [1, BD]]))
                return
            if jr == 0:
                src_t, src_off = tu_box['prev'], (BD - 1) * STW
            else:
                src_t, src_off = tu8, (jr - 1) * STW
            mo = mopool.tile([P, NHR * JW * BD * BD], F16, tag="mo",
                             name=f"mo{tau}_{jr}")
            # mo[hr, jcol, i, k] = ST[hr, jcol, k] * A[hr, i, k]
            if MULT_SPLIT_HR:
                for hr in range(NHR):
                    nc.vector.tensor_tensor(
                        bass.AP(mo.tensor, mo[:].offset + hr * JW * BD * BD,
                                [[NHR * JW * BD * BD, P], [BD * BD, JW],
                                 [BD, BD], [1, BD]]),
                        bass.AP(src_t.tensor,
                                src_t[:].offset + src_off + hr * JW * BD,
                                [[BD * STW, P], [BD, JW], [0, BD], [1, BD]]),
                        bass.AP(a8.tensor,
                                a8[:].offset + jr * NHR * BD * BD + hr * BD * BD,
                                [[BD * NHR * BD * BD, P], [0, JW],
                                 [BD, BD], [1, BD]]),
                        ALU.mult)
            else:
                nc.vector.tensor_tensor(
                    bass.AP(mo.tensor, mo[:].offset,
                            [[NHR * JW * BD * BD, P], [JW * BD * BD, NHR],
                             [BD * BD, JW], [BD, BD], [1, BD]]),
                    bass.AP(src_t.tensor, src_t[:].offset + src_off,
                            [[BD * STW, P], [JW * BD, NHR],
                             [BD, JW], [0, BD], [1, BD]]),
                    bass.AP(a8.tensor, a8[:].offset + jr * NHR * BD * BD,
                            [[BD * NHR * BD * BD, P], [BD * BD, NHR],
                             [0, JW], [BD, BD], [1, BD]]),
                    ALU.mult)
            m1 = mopool.tile([P, NHR * JW * BD * 4], F16, tag="m1",
                             name=f"m1{tau}_{jr}")
            nc.vector.tensor_tensor(
                m1[:],
                bass.AP(mo.tensor, mo[:].offset,
                        [[NHR * JW * BD * BD, P], [BD, NHR * JW * BD], [1, 4]]),
                bass.AP(mo.tensor, mo[:].offset + 4,
                        [[NHR * JW * BD * BD, P], [BD, NHR * JW * BD], [1, 4]]),
                ALU.add)
            m2 = mopool.tile([P, NHR * JW * BD * 2], F16, tag="m2",
                             name=f"m2{tau}_{jr}")
            nc.vector.tensor_tensor(
                m2[:],
                bass.AP(m1.tensor, m1[:].offset,
                        [[NHR * JW * BD * 4, P], [4, NHR * JW * BD], [1, 2]]),
                bass.AP(m1.tensor, m1[:].offset + 2,
                        [[NHR * JW * BD * 4, P], [4, NHR * JW * BD], [1, 2]]),
                ALU.add)
            nc.vector.tensor_tensor(
                bass.AP(tu8.tensor, tu8[:].offset + jr * STW,
                        [[BD * STW, P], [1, STW]]),
                bass.AP(m2.tensor, m2[:].offset,
                        [[NHR * JW * BD * 2, P], [2, STW]]),
                bass.AP(m2.tensor, m2[:].offset + 1,
                        [[NHR * JW * BD * 2, P], [2, STW]]),
                ALU.add)
            urow = bass.AP(tu8.tensor, tu8[:].offset + jr * STW + BD * BD,
                           [[BD * STW, P], [JW * BD, NHR], [1, BD]])
            nc.vector.tensor_tensor(
                urow, urow,
                bass.AP(v8.tensor, v8[:].offset + jr * NHR * BD,
                        [[BD * NHR * BD, P], [BD, NHR], [1, BD]]),
                ALU.add)

        def p1_tile(tau):
            if tau >= NT - 2:
                tu_box[f'tu8_{tau}'] = None  # filled below
            a8 = agpool.tile([P, BD, NHR * BD * BD], F16, tag="a8",
                             name=f"a8_{tau}")
            v8 = vgpool.tile([P, BD, NHR * BD], F16, tag="v8", name=f"v8_{tau}")
            geng = nc.gpsimd
            geng.dma_start(a8[:], bass.AP(
                a_dram, tau * P * NF, [[BD * 256, P], [1, BD * 256]]))
            geng.dma_start(v8[:], bass.AP(
                v_dram, tau * P * VF, [[BD * 32, P], [1, BD * 32]]))
            tu8 = tupool.tile([P, BD, STW], F16, tag="tu8", name=f"tu8_{tau}")
            for jr in range(BD):
                p1_step(tau, jr, a8, v8, tu8)
            tu_box['prev'] = tu8
            tu_box[f'tu8_{tau}'] = tu8
            nc.scalar.dma_start(
                bass.AP(tu_dram, tau * P * BD * STW,
                        [[BD * STW, P], [1, BD * STW]]),
                tu8[:])

        # ================= stage A =================
        SCHED = [4, 4, 4, 2, 2]  # tiles per q-group (tapered tail)
        t0 = 0
        for qi, nt_q in enumerate(SCHED):
            W = nt_q * P
            if qi == 0:
                xq = xq0
            else:
                xq = xpool.tile([P, 4, QT], BF16, tag="xq", name=f"xq{qi}")
                for ti in range(nt_q):
                    for k in range(4):
                        nc.sync.dma_start(
                            xq[:, k, bass.ts(ti, P)],
                            bass.AP(xT, k * P * SEQ + (t0 + ti) * 8,
                                    [[SEQ, P], [CHK, K], [1, 8]]))

            hid_t = hpool.tile([P, HID // P, QT], BF16, tag="hid",
                               name=f"hid{qi}")
            for m in range(HID // P):
                ps = l1ps.tile([P, QT], F32, tag="l1")
                for k in range(4):
                    nc.tensor.matmul(ps[:, :W], w1_s[:, k, bass.ts(m, P)],
                                     xq[:, k, :W], start=(k == 0),
                                     stop=(k == 3))
                nc.scalar.activation(hid_t[:, m, :W], ps[:, :W], AF.Relu,
                                     bias=b1_s[:, m:m + 1])

            hv_t = hvpool.tile([P, 4, QT], BF16, tag="hv", name=f"hv{qi}")
            for m in range(4):
                ps = l1ps.tile([P, QT], F32, tag="l1")
                for k in range(4):
                    nc.tensor.matmul(ps[:, :W], v1_s[:, k, bass.ts(m, P)],
                                     xq[:, k, :W], start=(k == 0),
                                     stop=(k == 3))
                nc.scalar.activation(hv_t[:, m, :W], ps[:, :W], AF.Relu,
                                     bias=c1_s[:, m:m + 1])

            # ---- L2 + per-slice norm ----
            for n in range(NSL):
                pss = [l2ps.tile([P, SL], F32, tag="l2",
                                 name=f"l2_{qi}_{n}_{i}")
                       for i in range(nt_q)]
                for ti in range(nt_q):
                    nc.tensor.matmul(pss[ti][:], ones_s[:1, :],
                                     b2_s[:1, bass.ts(n, SL)],
                                     start=True, stop=False)
                for kg in range(8):
                    w2s = w2pool.tile([P, 4, SL], BF16, tag="w2s",
                                      name=f"w2_{qi}_{n}_{kg}")
                    nc.sync.dma_start(w2s[:], bass.AP(
                        w2, kg * 4 * P * NF + n * SL,
                        [[NF, P], [P * NF, 4], [1, SL]]))
                    for k8 in range(4):
                        k = kg * 4 + k8
                        for ti in range(nt_q):
                            nc.tensor.matmul(
                                pss[ti][:], hid_t[:, k, bass.ts(ti, P)],
                                w2s[:, k8, :], start=False,
                                stop=(k == HID // P - 1))
                # psum-freeing copies first, then the norm chains
                blks = []
                for ti in range(nt_q):
                    blk = blkpool.tile([P, SL], F32, tag="blk",
                                       name=f"blk{qi}_{n}_{ti}")
                    nc.scalar.activation(blk[:], pss[ti][:], AF.Identity)
                    blks.append(blk)
                for ti in range(nt_q):
                    blk = blks[ti]
                    pw = pwpool.tile([P, SL], F32, tag="pw")
                    nc.scalar.activation(pw[:], blk[:], AF.Square)
                    nc.scalar.activation(pw[:], pw[:], AF.Ln)
                    nc.scalar.activation(pw[:], pw[:], AF.Exp, scale=0.6)
                    pst = smpool.tile([P, HSL * BD], F32, tag="pst")
                    nc.vector.tensor_reduce(
                        pst[:].rearrange("p (h j) -> p h j", h=HSL, j=BD),
                        bass.AP(pw.tensor, pw[:].offset,
                                [[SL, P], [64, HSL], [1, BD], [8, BD]]),
                        axis=mybir.AxisListType.X, op=ALU.add)
                    dm = smpool.tile([P, HSL], F32, tag="dm")
                    nc.vector.tensor_reduce(
                        dm[:].rearrange("p (h one) -> p h one", h=HSL, one=1),
                        pst[:].rearrange("p (h j) -> p h j", h=HSL, j=BD),
                        axis=mybir.AxisListType.X, op=ALU.max)
                    rc = smpool.tile([P, HSL], F32, tag="rc")
                    nc.scalar.activation(rc[:], dm[:], AF.Ln)
                    nc.scalar.activation(rc[:], rc[:], AF.Exp,
                                         scale=-1.0 / 1.2)
                    asl = aipool.tile([P, SL], F16, tag="ai",
                                      name=f"ai{qi}_{n}_{ti}")
                    nc.gpsimd.tensor_tensor(
                        asl[:].rearrange("p (h i k) -> p h i k",
                                         h=HSL, i=BD, k=BD),
                        blk[:].rearrange("p (h i k) -> p h i k",
                                         h=HSL, i=BD, k=BD),
                        bass.AP(rc.tensor, rc[:].offset,
                                [[HSL, P], [1, HSL], [0, BD], [0, BD]]),
                        ALU.mult)
                    tt = t0 + ti
                    weng = nc.gpsimd
                    for ho2 in range(2):
                        # dst (tau, c, ho=2n+ho2, jr=j, (hr,i,k))
                        weng.dma_start(
                            bass.AP(a_dram,
                                    tt * P * NF + (2 * n + ho2) * BD * 256,
                                    [[NHO * BD * 256, K], [256, BD],
                                     [1, 256]]),
                            bass.AP(asl.tensor, asl[:].offset + ho2 * 256,
                                    [[SL, P], [1, 256]]))

            for ti in range(nt_q):
                tt = t0 + ti
                psv = vps.tile([P, VF], F32, tag="v")
                nc.tensor.matmul(psv[:], ones_s[:1, :], c2_s[:1, :],
                                 start=True, stop=False)
                for k in range(4):
                    nc.tensor.matmul(psv[:], hv_t[:, k, bass.ts(ti, P)],
                                     v2_s[:, k, :], start=False,
                                     stop=(k == 3))
                vt = vtpool.tile([P, VF], F16, tag="vt")
                nc.scalar.activation(vt[:], psv[:], AF.Identity)
                veng = nc.gpsimd
                for ho in range(NHO):
                    veng.dma_start(
                        bass.AP(v_dram, tt * P * VF + ho * BD * 32,
                                [[NHO * BD * 32, K], [32, BD], [1, 32]]),
                        bass.AP(vt.tensor, vt[:].offset + ho * 32,
                                [[VF, P], [1, 32]]))

                p1_tile(tt)
            t0 += nt_q

        # ================= phase B =================
        # extract per-chunk summaries from tu8 of the last tile (SBUF->SBUF)
        tu_last = tu_box['prev']
        tub = scpool.tile([NHO, K * STW], F16, tag="tub")
        for c in range(K):
            qeng = nc.sync if c % 2 == 0 else nc.scalar
            qeng.dma_start(
                bass.AP(tub.tensor, tub[:].offset + c * STW,
                        [[K * STW, NHO], [1, STW]]),
                bass.AP(tu_last.tensor,
                        tu_last[:].offset + c * NHO * BD * STW
                        + (BD - 1) * STW,
                        [[BD * STW, NHO], [1, STW]]))
        sg = scpool.tile([NHO, (K + 1) * NHR * BD], F16, tag="sg")
        sc_t = scpool.tile([P, NHR * BD], F16, tag="sc")
        nc.vector.tensor_copy(sg[:, 0:NHR * BD], a0_s[:])
        nc.sync.dma_start(sc_t[0:NHO, :], sg[:, 0:NHR * BD])
        for c in range(K):
            mB = smpool.tile([NHO, NHR * BD * BD], F32, tag="mB", name=f"mB{c}")
            nc.vector.tensor_tensor(
                bass.AP(mB.tensor, mB[:].offset,
                        [[NHR * BD * BD, NHO], [BD * BD, NHR],
                         [BD, BD], [1, BD]]),
                bass.AP(tub.tensor, tub[:].offset + c * STW,
                        [[K * STW, NHO], [JW * BD, NHR], [BD, BD], [1, BD]]),
                bass.AP(sg.tensor, sg[:].offset + c * NHR * BD,
                        [[(K + 1) * NHR * BD, NHO], [BD, NHR],
                         [1, BD], [0, BD]]),
                ALU.mult)
            sn = smpool.tile([NHO, NHR * BD], F32, tag="sn", name=f"sn{c}")
            nc.vector.tensor_reduce(
                bass.AP(sn.tensor, sn[:].offset,
                        [[NHR * BD, NHO], [BD, NHR], [1, BD]]),
                bass.AP(mB.tensor, mB[:].offset,
                        [[NHR * BD * BD, NHO], [BD * BD, NHR],
                         [1, BD], [BD, BD]]),
                axis=mybir.AxisListType.X, op=ALU.add)
            nc.vector.tensor_tensor(
                bass.AP(sg.tensor, sg[:].offset + (c + 1) * NHR * BD,
                        [[(K + 1) * NHR * BD, NHO], [1, NHR * BD]]),
                bass.AP(sn.tensor, sn[:].offset,
                        [[NHR * BD, NHO], [1, NHR * BD]]),
                bass.AP(tub.tensor, tub[:].offset + c * STW + BD * BD,
                        [[K * STW, NHO], [JW * BD, NHR], [1, BD]]),
                ALU.add)
            if c + 1 < K:
                nc.sync.dma_start(
                    sc_t[(c + 1) * NHO:(c + 2) * NHO, :],
                    sg[:, (c + 1) * NHR * BD:(c + 2) * NHR * BD])


        sexp = scpool.tile([P, NHR * BD * BD], F16, tag="sexp")
        nc.vector.tensor_copy(
            bass.AP(sexp.tensor, sexp[:].offset,
                    [[NHR * BD * BD, P], [BD, NHR * BD], [1, BD]]),
            bass.AP(sc_t.tensor, sc_t[:].offset,
                    [[NHR * BD, P], [1, NHR * BD], [0, BD]]))

        # ================= phase C =================
        NTD = NT - 4  # DVE handles tau 0..11, Pool 12..15
        s_outA = scpool.tile([P, NTD * BD * NHR * BD], F32, tag="soutA")
        s_outB = scpool.tile([P, 4 * BD * NHR * BD], F32, tag="soutB")
        PCW = BD * NHR * BD * BD  # 2048
        for tau in range(NT):
            on_pool = (tau >= NTD)
            eng = nc.gpsimd if on_pool else nc.vector
            if tau >= NT - 2 and f'tu8_{tau}' in tu_box:
                tuc = tu_box[f'tu8_{tau}']  # still live in SBUF
            else:
                tpool = tcpoolB if on_pool else tcpool
                tuc = tpool.tile([P, BD, STW], F16, tag="tuc",
                                 name=f"tuc{tau}")
                nc.sync.dma_start(tuc[:], bass.AP(
                    tu_dram, tau * P * BD * STW,
                    [[BD * STW, P], [1, BD * STW]]))
            if on_pool:
                m = pcpoolB.tile([P, PCW], F16, tag="pcm", name=f"pcm{tau}")
                m1 = pcpoolB.tile([P, PCW // 2], F16, tag="pcm1",
                                  name=f"pcm1{tau}")
                m2 = pcpoolB.tile([P, PCW // 4], F16, tag="pcm2",
                                  name=f"pcm2{tau}")
                m3 = pcpoolB.tile([P, PCW // 8], F16, tag="pcm3",
                                  name=f"pcm3{tau}")
            else:
                m = mopool.tile([P, NHR * JW * BD * BD], F16, tag="mo",
                                name=f"pcm{tau}")
                m1 = mopool.tile([P, NHR * JW * BD * 4], F16, tag="m1",
                                 name=f"pcm1{tau}")
                m2 = mopool.tile([P, NHR * JW * BD * 2], F16, tag="m2",
                                 name=f"pcm2{tau}")
                m3 = mopool.tile([P, NHR * JW * BD * BD], F16, tag="mo",
                                 name=f"pcm3{tau}")
            MP = m[:].ap[0][0]
            M1P = m1[:].ap[0][0]
            M2P = m2[:].ap[0][0]
            M3P = m3[:].ap[0][0]
            # m[jr, hr, (k,i)] = ST[jr,hr,jcol=k,i] * sigma_exp[hr,(k,i)]
            eng.tensor_tensor(
                bass.AP(m.tensor, m[:].offset,
                        [[MP, P], [NHR * BD * BD, BD],
                         [BD * BD, NHR], [1, BD * BD]]),
                bass.AP(tuc.tensor, tuc[:].offset,
                        [[BD * STW, P], [STW, BD], [JW * BD, NHR],
                         [1, BD * BD]]),
                bass.AP(sexp.tensor, sexp[:].offset,
                        [[NHR * BD * BD, P], [0, BD], [BD * BD, NHR],
                         [1, BD * BD]]),
                ALU.mult)
            eng.tensor_tensor(
                bass.AP(m1.tensor, m1[:].offset, [[M1P, P], [1, PCW // 2]]),
                bass.AP(m.tensor, m[:].offset,
                        [[MP, P], [BD * BD, BD * NHR], [1, 4 * BD]]),
                bass.AP(m.tensor, m[:].offset + 4 * BD,
                        [[MP, P], [BD * BD, BD * NHR], [1, 4 * BD]]),
                ALU.add)
            eng.tensor_tensor(
                bass.AP(m2.tensor, m2[:].offset, [[M2P, P], [1, PCW // 4]]),
                bass.AP(m1.tensor, m1[:].offset,
                        [[M1P, P], [4 * BD, BD * NHR], [1, 2 * BD]]),
                bass.AP(m1.tensor, m1[:].offset + 2 * BD,
                        [[M1P, P], [4 * BD, BD * NHR], [1, 2 * BD]]),
                ALU.add)
            eng.tensor_tensor(
                bass.AP(m3.tensor, m3[:].offset, [[M3P, P], [1, PCW // 8]]),
                bass.AP(m2.tensor, m2[:].offset,
                        [[M2P, P], [2 * BD, BD * NHR], [1, BD]]),
                bass.AP(m2.tensor, m2[:].offset + BD,
                        [[M2P, P], [2 * BD, BD * NHR], [1, BD]]),
                ALU.add)
            # s = m3 + u  -> s_out half (f32)
            s_o = s_outB if on_pool else s_outA
            s_off = (tau - NTD if on_pool else tau) * BD * NHR * BD
            eng.tensor_tensor(
                bass.AP(s_o.tensor, s_o[:].offset + s_off,
                        [[s_o[:].ap[0][0], P], [1, BD * NHR * BD]]),
                bass.AP(m3.tensor, m3[:].offset, [[M3P, P], [1, PCW // 8]]),
                bass.AP(tuc.tensor, tuc[:].offset + BD * BD,
                        [[BD * STW, P], [STW, BD], [JW * BD, NHR], [1, BD]]),
                ALU.add)

        # ---- output: s_out halves [(c,ho), (pos, hr, i)] -> out[t, vf] ----
        for c in range(K):
            nc.sync.dma_start(
                bass.AP(out, c * CHK * VF,
                        [[NHR * BD, NHO], [VF, NTD * BD], [1, NHR * BD]]),
                bass.AP(s_outA.tensor,
                        s_outA[c * NHO:(c + 1) * NHO, :].offset,
                        [[NTD * BD * NHR * BD, NHO], [NHR * BD, NTD * BD],
                         [1, NHR * BD]]))
            nc.scalar.dma_start(
                bass.AP(out, (c * CHK + NTD * BD) * VF,
                        [[NHR * BD, NHO], [VF, 4 * BD], [1, NHR * BD]]),
                bass.AP(s_outB.tensor,
                        s_outB[c * NHO:(c + 1) * NHO, :].offset,
                        [[4 * BD * NHR * BD, NHO], [NHR * BD, 4 * BD],
                         [1, NHR * BD]]))

    nc.compile()
    return nc


# ---------------- host side ----------------

_NC_CACHE = {}


def _get_nc(TOK=SEQ):
    if TOK not in _NC_CACHE:
        _NC_CACHE[TOK] = build_nc()
    return _NC_CACHE[TOK]


def prep_shared(W1, b1, W2, b2, V1, c1, V2, c2, a0):
    bf = ml_dtypes.bfloat16
    W2r = np.asarray(W2).reshape(H, BD, BD, HID)
    W2c = (W2r - W2r.mean(axis=1, keepdims=True)).reshape(H * BD * BD, HID)
    b2r = np.asarray(b2).reshape(H, BD, BD)
    b2c = (b2r - b2r.mean(axis=1, keepdims=True)).reshape(-1)
    shared = {
        "w1": np.ascontiguousarray(np.asarray(W1).T).astype(bf),
        "b1": np.asarray(b1).reshape(HID, 1).astype(np.float32),
        "v1": np.ascontiguousarray(np.asarray(V1).T).astype(bf),
        "c1": np.asarray(c1).reshape(EMB, 1).astype(np.float32),
    }
    halves = []
    for half in range(2):
        rsl = slice(half * NF, (half + 1) * NF)
        vsl = slice(half * VF, (half + 1) * VF)
        hsl = slice(half * HL, (half + 1) * HL)
        a0h = np.asarray(a0)[0, hsl]                       # [32, 8]
        a0p = a0h.reshape(NHO, NHR * BD)                   # [8, 32]
        halves.append({
            "w2": np.ascontiguousarray(W2c[rsl].T).astype(bf),
            "b2": b2c[rsl].reshape(1, NF).astype(bf),
            "v2": np.ascontiguousarray(np.asarray(V2)[vsl].T).astype(bf),
            "c2": np.asarray(c2)[vsl].reshape(1, VF).astype(bf),
            "a0": a0p.astype(np.float16),
        })
    return shared, halves


def make_in_maps(x, W1, b1, W2, b2, V1, c1, V2, c2, a0):
    shared, halves = prep_shared(W1, b1, W2, b2, V1, c1, V2, c2, a0)
    bf = ml_dtypes.bfloat16
    in_maps = []
    for core in range(N_CORES):
        b, half = core // 2, core % 2
        m = dict(shared)
        m.update(halves[half])
        m["xT"] = np.ascontiguousarray(np.asarray(x)[b].T).astype(bf)
        in_maps.append(m)
    return in_maps


def kernel(x, W1, b1, W2, b2, V1, c1, V2, c2, a0):
    from concourse import bass_utils
    nc = _get_nc(SEQ)
    in_maps = make_in_maps(x, W1, b1, W2, b2, V1, c1, V2, c2, a0)
    res = bass_utils.run_bass_kernel_spmd(nc, in_maps,
                                          core_ids=list(range(N_CORES)))
    out = np.zeros((BS, SEQ, EMB), np.float32)
    for core in range(N_CORES):
        b, half = core // 2, core % 2
        out[b, :, half * VF:(half + 1) * VF] = res.results[core]["out"]
    return out
